# revision 17
# baseline (speedup 1.0000x reference)
"""Trainium2 Bass kernel: 2-layer GCN (GCNConv -> ReLU -> GCNConv -> Linear).

Strategy (8 NeuronCores, SPMD), v3 "dense edge-slab" design:
  - Destination-node sharding; nodes assigned to (core, lane) by a
    degree-sorted serpentine so per-(core,tile) edge counts match across
    cores (minimal static padding).
  - 3 launches with host-side exchange of the small activation tables:
      A: H1 = X @ W1                      (row-sharded dense matmul)
      B: MP1 + b1 + ReLU, then @ (W2 Wp) -> T2   (feature-major out)
      C: MP2 + bpp -> y                   (feature-major out)
  - Message passing consumes a host-expanded *dense edge slab*: for each
    128-edge chunk the 128 source rows are laid out contiguously in DRAM
    (edge order, dest-sorted, self-loops included as ordinary edges).  The
    device streams the slab at full DMA bandwidth -- no dma_gather, no
    GpSimd descriptor generation (the baseline bottleneck).
  - Transposed MP matmul: out[f, lane] += slab_chunk[slot, f]^T-contracted
    with wsl[slot, lane-window].  The destination window lives in the PSUM
    *free* dim, so any [mn..mx] window is legal (single matmul per chunk).
    The first chunk of each tile stores a full 128-wide weight block and
    runs with start=True (PSUM zeroing without a bias bracket).
  - Biases ride the Scalar-engine activation (per-partition bias AP), not
    PE matmuls.  Outputs are staged in SBUF and written in 8-tile batches.
  - All matmul operands bf16 (fp32 PSUM accumulation); final output fp32
    (bf16 on the wire, upcast on host).
"""

from contextlib import ExitStack
from dataclasses import dataclass, field

import numpy as np
import ml_dtypes

BF16 = ml_dtypes.bfloat16
E4M3 = ml_dtypes.float8_e4m3fn
FP32 = np.float32


# ---------------------------------------------------------------- config

@dataclass
class Cfg:
    N: int = 50000
    IN_DIM: int = 512
    HID: int = 256
    OUT: int = 128
    NCORES: int = 8
    BLK_B: int = 32       # slab chunks per stream DMA, launch B (16KB/part)
    BLK_C: int = 64       # launch C (16KB/part)
    TB_A: int = 8         # x tiles per stream DMA, launch A (8KB/part)
    GRP: int = 8          # output tiles per batched store
    MASS_FRAC: float = 0.17   # fraction of sum(norm^2) carried by fp8 edges

    ND: int = field(init=False)
    NTILES: int = field(init=False)
    NP: int = field(init=False)

    def __post_init__(self):
        self.ND = self.N // self.NCORES
        self.NTILES = (self.ND + 127) // 128
        self.NP = self.NTILES * 128


# ---------------------------------------------------------------- planner

class Plan:
    """Static (cross-core identical) chunk geometry + per-core data."""

    def __init__(self, cfg: Cfg, edge_index, edge_weight):
        self.cfg = cfg
        N, ND, NP, NT = cfg.N, cfg.ND, cfg.NP, cfg.NTILES
        NC = cfg.NCORES

        # --- gcn_norm with self loops (kept as ordinary edges)
        row = np.concatenate([np.asarray(edge_index[0], np.int64),
                              np.arange(N, dtype=np.int64)])
        col = np.concatenate([np.asarray(edge_index[1], np.int64),
                              np.arange(N, dtype=np.int64)])
        w = np.concatenate([np.asarray(edge_weight, np.float64),
                            np.ones(N, np.float64)])
        deg = np.zeros(N, np.float64)
        np.add.at(deg, col, w)
        dinv = np.where(deg > 0, 1.0 / np.sqrt(deg), 0.0)
        nrm = (dinv[row] * w * dinv[col]).astype(np.float32)

        # --- degree-sorted serpentine node -> (core, lane)
        degi = np.bincount(col, minlength=N)
        ranks = np.argsort(-degi, kind="stable")    # rank r -> node
        r = np.arange(N)
        blk = r // NC
        corepos = np.where(blk % 2 == 0, r % NC, NC - 1 - (r % NC))
        lane_global = np.empty(N, np.int64)         # node -> core*NP + lane
        lane_global[ranks] = corepos * NP + blk
        self.nodes = []                             # per core: lane -> node id
        for k in range(NC):
            nk = np.empty(ND, np.int64)
            sel = corepos == k
            nk[blk[sel]] = ranks[sel]
            self.nodes.append(nk)

        # --- edge geometry, dest-sorted
        src_row = lane_global[row]                  # table row of the source
        dst = lane_global[col]
        dst_core = dst // NP
        dlane = dst % NP
        dtile = dlane // 128
        dl = dlane % 128

        order = np.lexsort((dl, dtile, dst_core))
        so_core = dst_core[order]
        so_tile = dtile[order]
        so_lane = dl[order]
        so_src = src_row[order]
        so_w = nrm[order]

        # --- fp8 / bf16 split: the low-|norm| edges carrying MASS_FRAC of
        # sum(norm^2) ride in fp8 slabs (per-source-row scaling folded into
        # their weight slab).  Self-loops stay bf16 so every (core,tile) has
        # a bf16 first chunk.
        so_self = np.concatenate([row[:len(row) - N] == col[:len(row) - N],
                                  np.ones(N, bool)])[order]
        w2 = so_w.astype(np.float64) ** 2
        ow = np.argsort(np.abs(so_w), kind="stable")
        cmass = np.cumsum(w2[ow])
        nf8 = int(np.searchsorted(cmass / max(cmass[-1], 1e-30),
                                  cfg.MASS_FRAC))
        f8 = np.zeros(len(so_w), bool)
        f8[ow[:nf8]] = True
        f8 &= ~self._selfmask(row, col, N, order)

        def geom(sel, force_first_full):
            seg_s = so_core[sel] * NT + so_tile[sel]
            cnt_s = np.bincount(seg_s, minlength=NC * NT).reshape(NC, NT)
            CH = (-(-cnt_s // 128)).max(axis=0)
            base = np.concatenate([[0], np.cumsum(CH)])
            TOT = int(base[-1])
            seg_start = np.concatenate(
                [[0], np.cumsum(np.bincount(seg_s, minlength=NC * NT))])[:-1]
            rank = np.arange(sel.sum()) - seg_start[seg_s]
            chunk = base[so_tile[sel]] + rank // 128
            slot = rank % 128
            mn = np.full(max(TOT, 1), 128, np.int64)
            mx = np.full(max(TOT, 1), -1, np.int64)
            lanes = so_lane[sel]
            np.minimum.at(mn, chunk, lanes)
            np.maximum.at(mx, chunk, lanes)
            empty = mx < 0
            mn[empty] = 0
            mx[empty] = mn[empty] - 1
            if force_first_full and TOT:
                first = base[:-1]
                mn[first] = 0
                mx[first] = 127
            span = mx - mn + 1
            off = np.concatenate([[0], np.cumsum(span)])
            return dict(CH=CH, base=base, TOT=TOT, chunk=chunk, slot=slot,
                        mn=mn, span=span, off=off,
                        SLAB=max(int(off[-1]), 1))

        gb = geom(~f8, True)
        g8 = geom(f8, False)
        self.gb, self.g8 = gb, g8
        self.TOTCH, self.SLAB = gb["TOT"], gb["SLAB"]
        self.TOTCH8, self.SLAB8 = max(g8["TOT"], 1), g8["SLAB"]

        # --- per-core arrays
        self.wsl = []       # bf16 one-hot*norm blocks [128, SLAB]
        self.pos = []       # bf16 edge -> slab position
        self.srcrow = []
        self.pos8 = []      # fp8 edge -> slab8 position
        self.srcrow8 = []
        self.w8 = []        # fp8 edge norms
        self.wcol8 = []     # fp8 edge -> (slot, wsl8 column)
        kb = so_core[~f8]
        k8 = so_core[f8]
        srcb, src8 = so_src[~f8], so_src[f8]
        laneb, lane8 = so_lane[~f8], so_lane[f8]
        wb, w8v = so_w[~f8], so_w[f8]
        for k in range(NC):
            m = kb == k
            kchunk, kslot = gb["chunk"][m], gb["slot"][m]
            wsl = np.zeros((128, self.SLAB), np.float32)
            wsl[kslot, gb["off"][kchunk] + (laneb[m] - gb["mn"][kchunk])] = wb[m]
            self.wsl.append(wsl.astype(BF16))
            self.pos.append(kslot * self.TOTCH + kchunk)
            self.srcrow.append(srcb[m])

            m8 = k8 == k
            kchunk8, kslot8 = g8["chunk"][m8], g8["slot"][m8]
            self.pos8.append(kslot8 * self.TOTCH8 + kchunk8)
            self.srcrow8.append(src8[m8])
            self.w8.append(w8v[m8].astype(np.float64))
            self.wcol8.append((kslot8,
                               g8["off"][kchunk8]
                               + (lane8[m8] - g8["mn"][kchunk8])))

    @staticmethod
    def _selfmask(row, col, N, order):
        m = np.zeros(len(row), bool)
        m[len(row) - N:] = True
        return m[order]

    @staticmethod
    def row_scales(tab: np.ndarray) -> np.ndarray:
        mx = np.maximum(np.abs(tab.astype(np.float32)).max(axis=1), 1e-20)
        return (240.0 / mx).astype(np.float32)

    def build_slab(self, k: int, tab: np.ndarray) -> np.ndarray:
        """Dense edge-ordered bf16 slab [128, TOTCH*F] for core k."""
        F = tab.shape[1]
        flat = np.zeros((128 * self.TOTCH, F), BF16)
        flat[self.pos[k]] = tab[self.srcrow[k]]
        return flat.reshape(128, self.TOTCH * F)

    def build_slab8(self, k: int, tab: np.ndarray, s: np.ndarray):
        """Dense edge-ordered fp8 slab [128, TOTCH8*F] (rows scaled by s)."""
        F = tab.shape[1]
        flat = np.zeros((128 * self.TOTCH8, F), E4M3)
        src = self.srcrow8[k]
        flat[self.pos8[k]] = (tab[src].astype(np.float32)
                              * s[src][:, None]).astype(E4M3)
        return flat.reshape(128, self.TOTCH8 * F)

    def build_wsl8(self, k: int, s: np.ndarray) -> np.ndarray:
        arr = np.zeros((128, self.SLAB8), np.float32)
        kslot, kcol = self.wcol8[k]
        arr[kslot, kcol] = self.w8[k] / s[self.srcrow8[k]]
        return arr.astype(BF16)


# ---------------------------------------------------------------- bass builders

def _build_l1(cfg: Cfg):
    """H1 = X @ W1, feature-major output (two halves h1a/h1b [128, NP]).
    xt lives fully in SBUF (12.5KB/part); few, large DMAs -- the SP
    sequencer's ~0.6us per dma_start was the previous bottleneck."""
    import concourse.bacc as bacc
    import concourse.mybir as mybir
    import concourse.tile as tile

    dt = mybir.dt
    nc = bacc.Bacc(None, target_bir_lowering=False, num_swdge_queues=4)
    KCH = cfg.IN_DIM // 128
    G = 4                                   # tiles per matmul (512 lanes)
    NG = -(-cfg.NTILES // G)
    L = G * 128
    SG = 4                                  # matmul groups per output stage
    # c-major x^T: block c is x[:, c*128:(c+1)*128].T laid out [128, NP]
    xt = nc.dram_tensor("xt", [128, KCH * cfg.NP], dt.bfloat16,
                        kind="ExternalInput")
    w1 = nc.dram_tensor("w1", [128, KCH * cfg.HID], dt.bfloat16,
                        kind="ExternalInput")
    outs_d = [nc.dram_tensor(f"h1{h}", [128, cfg.NP], dt.bfloat16,
                             kind="ExternalOutput") for h in range(2)]

    with tile.TileContext(nc) as tc, ExitStack() as ctx:
        consts = ctx.enter_context(tc.tile_pool(name="consts", bufs=1))
        stg = ctx.enter_context(tc.tile_pool(name="stg", bufs=2))
        pools = [ctx.enter_context(tc.tile_pool(name=f"psl{h}", bufs=3,
                                                space="PSUM"))
                 for h in range(2)]

        w1_sb = consts.tile([128, KCH * cfg.HID], dt.bfloat16, tag="w1")
        xt_sb = consts.tile([128, KCH * cfg.NP], dt.bfloat16, tag="xt")
        # interleave c-quarters so early (g, all-c) windows complete fast
        Q = 8
        qs = cfg.NP // Q
        nc.sync.dma_start(w1_sb[:], w1[:])
        for q in range(Q):
            for c in range(KCH):
                nc.sync.dma_start(
                    xt_sb[:, c * cfg.NP + q * qs: c * cfg.NP + (q + 1) * qs],
                    xt[:, c * cfg.NP + q * qs: c * cfg.NP + (q + 1) * qs])

        stages = [None, None]
        for g in range(NG):
            l0 = g * L
            l1 = min(cfg.NP, l0 + L)
            ll = l1 - l0
            sgi = g % SG
            if sgi == 0:
                nst = min(SG * L, cfg.NP - g * L)
                stages = [stg.tile([128, nst], dt.bfloat16, name="ostg")
                          for _ in range(2)]
            for h in range(2):
                ps = pools[h].tile([128, L], dt.float32, name="psl")
                for c in range(KCH):
                    nc.tensor.matmul(
                        ps[:, :ll],
                        w1_sb[:, c * cfg.HID + h * 128:
                              c * cfg.HID + (h + 1) * 128],
                        xt_sb[:, c * cfg.NP + l0: c * cfg.NP + l1],
                        start=(c == 0), stop=(c == KCH - 1),
                    )
                if h == 0:
                    nc.scalar.activation(
                        stages[h][:, sgi * L: sgi * L + ll], ps[:, :ll],
                        mybir.ActivationFunctionType.Copy)
                else:
                    nc.vector.tensor_copy(
                        stages[h][:, sgi * L: sgi * L + ll], ps[:, :ll])
            if sgi == SG - 1 or g == NG - 1:
                g0 = (g // SG) * SG * L
                for h in range(2):
                    nc.sync.dma_start(outs_d[h][:, g0:l1],
                                      stages[h][:, : l1 - g0])
    nc.finalize()
    return nc


def _build_mp(cfg: Cfg, plan: Plan, layer2: bool):
    """Transposed-MP launch.
    layer2: MP1 + b1 + ReLU + @W2p -> T2 [128, NP] bf16 (feature-major).
    else:   MP2 + bpp -> y [128, NP] bf16 (feature-major)."""
    import concourse.bacc as bacc
    import concourse.mybir as mybir
    import concourse.tile as tile

    dt = mybir.dt
    F = cfg.HID if layer2 else cfg.OUT          # slab feature width
    FCH = F // 128                              # psum column-tiles (2 or 1)
    BLK = cfg.BLK_B if layer2 else cfg.BLK_C
    BLK8 = 2 * BLK            # fp8 chunks are half the bytes
    GRP = cfg.GRP
    nc = bacc.Bacc(None, target_bir_lowering=False, num_swdge_queues=4)

    slab = nc.dram_tensor("slab", [128, plan.TOTCH * F], dt.bfloat16,
                          kind="ExternalInput")
    slab8 = nc.dram_tensor("slab8", [128, plan.TOTCH8 * F], dt.float8e4,
                           kind="ExternalInput")
    wsl = nc.dram_tensor("wsl", [128, plan.SLAB], dt.bfloat16,
                         kind="ExternalInput")
    wsl8 = nc.dram_tensor("wsl8", [128, plan.SLAB8], dt.bfloat16,
                          kind="ExternalInput")
    bvec = nc.dram_tensor("bvec", [128, FCH], dt.float32,
                          kind="ExternalInput")
    if layer2:
        w2p = nc.dram_tensor("w2p", [128, FCH * cfg.OUT], dt.bfloat16,
                             kind="ExternalInput")
    out = nc.dram_tensor("out", [128, cfg.NTILES * 128], dt.bfloat16,
                         kind="ExternalOutput")

    # split the wsl load at tile boundaries so early tiles' matmuls don't
    # wait on the whole slab-weight transfer
    nsplit = 4
    wcuts = [0]
    for i in range(1, nsplit):
        t = (cfg.NTILES * i) // nsplit
        wcuts.append(int(plan.gb["off"][plan.gb["base"][t]]))
    wcuts.append(plan.SLAB)

    with tile.TileContext(nc) as tc, ExitStack() as ctx:
        consts = ctx.enter_context(tc.tile_pool(name="consts", bufs=1))
        sstr = ctx.enter_context(tc.tile_pool(name="sstr", bufs=5))
        work = ctx.enter_context(tc.tile_pool(name="work", bufs=4))
        stg = ctx.enter_context(tc.tile_pool(name="stg", bufs=2))
        pools = [ctx.enter_context(tc.tile_pool(name=f"ps{c}",
                                                bufs=(3 if layer2 else 6),
                                                space="PSUM"))
                 for c in range(FCH)]
        if layer2:
            ps2p = ctx.enter_context(tc.tile_pool(name="ps2", bufs=2,
                                                  space="PSUM"))

        s8str = ctx.enter_context(tc.tile_pool(name="s8str", bufs=3))
        wsl_sb = consts.tile([128, plan.SLAB], dt.bfloat16, tag="wsl")
        wsl8_sb = consts.tile([128, plan.SLAB8], dt.bfloat16, tag="wsl8")
        bvec_sb = consts.tile([128, FCH], dt.float32, tag="bvec")
        if layer2:
            w2p_sb = consts.tile([128, FCH * cfg.OUT], dt.bfloat16, tag="w2p")

        stiles = {}

        def sblock(b):
            if b not in stiles:
                t = sstr.tile([128, BLK * F], dt.bfloat16, name="sb")
                c0 = b * BLK * F
                c1 = min(plan.TOTCH * F, c0 + BLK * F)
                nc.sync.dma_start(t[:, : c1 - c0], slab[:, c0:c1])
                stiles[b] = t
            return stiles[b]

        s8tiles = {}

        def s8block(b):
            if b not in s8tiles:
                t = s8str.tile([128, BLK8 * F], dt.float8e4, name="s8b")
                c0 = b * BLK8 * F
                c1 = min(plan.TOTCH8 * F, c0 + BLK8 * F)
                nc.sync.dma_start(t[:, : c1 - c0], slab8[:, c0:c1])
                s8tiles[b] = t
            return s8tiles[b]

        sblock(0)                 # slab block 0 first on the DMA queue
        for i in range(nsplit):
            nc.sync.dma_start(wsl_sb[:, wcuts[i]:wcuts[i + 1]],
                              wsl[:, wcuts[i]:wcuts[i + 1]])
        h8 = plan.SLAB8 // 2
        nc.sync.dma_start(wsl8_sb[:, :h8], wsl8[:, :h8])
        nc.sync.dma_start(wsl8_sb[:, h8:], wsl8[:, h8:])
        nc.sync.dma_start(bvec_sb[:], bvec[:])
        if layer2:
            nc.sync.dma_start(w2p_sb[:], w2p[:])

        stage = None
        for t in range(cfg.NTILES):
            g = t % GRP
            if g == 0:
                ntg = min(GRP, cfg.NTILES - t)
                stage = stg.tile([128, ntg * 128], dt.bfloat16,
                                 name="stage")
            pss = [pools[c].tile([128, 128], dt.float32, name=f"pst")
                   for c in range(FCH)]
            gb, g8 = plan.gb, plan.g8
            j0, j1 = int(gb["base"][t]), int(gb["base"][t + 1])
            e0, e1 = int(g8["base"][t]), int(g8["base"][t + 1])
            work_items = [(False, j) for j in range(j0, j1)
                          if gb["span"][j] > 0]
            work_items += [(True, j) for j in range(e0, e1)
                           if g8["span"][j] > 0]
            for wi, (is8, j) in enumerate(work_items):
                gg = g8 if is8 else gb
                sp = int(gg["span"][j])
                o0 = int(gg["off"][j])
                m0 = int(gg["mn"][j])
                if is8:
                    st = s8block(j // BLK8)
                    soff = (j % BLK8) * F
                    wtile = wsl8_sb
                else:
                    st = sblock(j // BLK)
                    soff = (j % BLK) * F
                    wtile = wsl_sb
                for c in range(FCH):
                    nc.tensor.matmul(
                        pss[c][:, m0:m0 + sp],
                        st[:, soff + c * 128: soff + (c + 1) * 128],
                        wtile[:, o0:o0 + sp],
                        start=(wi == 0), stop=(wi == len(work_items) - 1),
                        skip_group_check=True,
                    )

            if layer2:
                acts = []
                for c in range(FCH):
                    a = work.tile([128, 128], dt.bfloat16, name="act")
                    nc.scalar.activation(a[:], pss[c][:],
                                         mybir.ActivationFunctionType.Relu,
                                         bias=bvec_sb[:, c:c + 1])
                    acts.append(a)
                ps2 = ps2p.tile([128, cfg.OUT], dt.float32)
                for c in range(FCH):
                    nc.tensor.matmul(ps2[:],
                                     w2p_sb[:, c * cfg.OUT:(c + 1) * cfg.OUT],
                                     acts[c][:],
                                     start=(c == 0), stop=(c == FCH - 1))
                nc.scalar.activation(stage[:, g * 128:(g + 1) * 128], ps2[:],
                                     mybir.ActivationFunctionType.Copy)
            else:
                nc.scalar.add(stage[:, g * 128:(g + 1) * 128], pss[0][:],
                              bvec_sb[:, 0:1])

            if g == GRP - 1 or t == cfg.NTILES - 1:
                t0 = t - g
                nc.sync.dma_start(out[:, t0 * 128:(t + 1) * 128],
                                  stage[:, :(g + 1) * 128])

    nc.finalize()
    return nc


# ---------------------------------------------------------------- host packing

def _pack_l1_inputs(cfg: Cfg, plan: Plan, x, W1):
    KCH = cfg.IN_DIM // 128
    w1r = np.zeros((128, KCH * cfg.HID), BF16)
    for c in range(KCH):
        w1r[:, c * cfg.HID:(c + 1) * cfg.HID] = \
            W1[c * 128:(c + 1) * 128, :].astype(BF16)
    maps = []
    for k in range(cfg.NCORES):
        xs = np.zeros((cfg.NP, cfg.IN_DIM), np.float32)
        xs[:cfg.ND] = x[plan.nodes[k]]
        xtr = np.zeros((128, KCH * cfg.NP), BF16)
        for c in range(KCH):
            xtr[:, c * cfg.NP:(c + 1) * cfg.NP] = \
                xs[:, c * 128:(c + 1) * 128].T.astype(BF16)
        maps.append({"xt": xtr, "w1": w1r})
    return maps


def _pack_mp_inputs(cfg: Cfg, plan: Plan, table, Wn, b, layer2):
    F = cfg.HID if layer2 else cfg.OUT
    FCH = F // 128
    bvec = np.zeros((128, FCH), np.float32)
    for c in range(FCH):
        bvec[:, c] = b[c * 128:(c + 1) * 128]
    scales = Plan.row_scales(table)
    maps = []
    for k in range(cfg.NCORES):
        m = {
            "slab": plan.build_slab(k, table),
            "slab8": plan.build_slab8(k, table, scales),
            "wsl": plan.wsl[k],
            "wsl8": plan.build_wsl8(k, scales),
            "bvec": bvec,
        }
        if layer2:
            wnr = np.zeros((128, FCH * cfg.OUT), BF16)
            for c in range(FCH):
                wnr[:, c * cfg.OUT:(c + 1) * cfg.OUT] = \
                    Wn[c * 128:(c + 1) * 128, :].astype(BF16)
            m["w2p"] = wnr
        maps.append(m)
    return maps


# ---------------------------------------------------------------- driver

def _run(nc, in_maps, cfg, trace=False):
    from concourse.bass_utils import run_bass_kernel_spmd
    res = run_bass_kernel_spmd(nc, in_maps, list(range(cfg.NCORES)), trace=trace)
    return res


def kernel_run(inputs, cfg=None, trace=False, sim=False):
    cfg = cfg or Cfg()
    x = np.asarray(inputs["x"], np.float32)
    plan = Plan(cfg, np.asarray(inputs["edge_index"]),
                np.asarray(inputs["edge_weight"], np.float32))
    W1 = np.asarray(inputs["W1"], np.float32)
    b1 = np.asarray(inputs["b1"], np.float32)
    W2 = np.asarray(inputs["W2"], np.float32)
    b2 = np.asarray(inputs["b2"], np.float32)
    Wp = np.asarray(inputs["Wp"], np.float32)
    bp = np.asarray(inputs["bp"], np.float32)

    results = []

    def run(build, maps, outname):
        nc = build()
        if sim:
            from concourse.bass_interp import CoreSim
            outs = []
            for k in range(cfg.NCORES):
                s = CoreSim(nc)
                for name, arr in maps[k].items():
                    s.tensor(name)[:] = arr
                s.simulate()
                outs.append({outname: s.tensor(outname).copy()})
            results.append(None)
            return outs
        r = _run(nc, maps, cfg, trace=trace)
        results.append(r)
        return r.results

    # fold the post-projection into layer 2: A(relu1@W2)@Wp = A(relu1@(W2@Wp))
    W2p = (W2 @ Wp).astype(np.float32)
    bpp = (b2 @ Wp + bp).astype(np.float32)

    def as_bf16(a):
        a = np.asarray(a)
        return a if a.dtype == BF16 else a.view(BF16)

    r1 = run(lambda: _build_l1(cfg), _pack_l1_inputs(cfg, plan, x, W1), "h1")
    T1 = np.concatenate(
        [np.concatenate([as_bf16(r["h10"]).T, as_bf16(r["h11"]).T], axis=1)
         for r in r1], axis=0)

    r2 = run(lambda: _build_mp(cfg, plan, True),
             _pack_mp_inputs(cfg, plan, T1, W2p, b1, True), "out")
    # feature-major [128, NP] -> row-major table [NCORES*NP, 128]
    T2 = np.concatenate([as_bf16(r["out"]).T for r in r2], axis=0)

    r3 = run(lambda: _build_mp(cfg, plan, False),
             _pack_mp_inputs(cfg, plan, T2, None, bpp, False), "out")

    y = np.empty((cfg.N, cfg.OUT), np.float32)
    for k in range(cfg.NCORES):
        shard = as_bf16(r3[k]["out"]).T.astype(np.float32)   # [NP, OUT]
        y[plan.nodes[k]] = shard[:cfg.ND]
    return y, results


def kernel(**inputs):
    y, _ = kernel_run(inputs)
    return y


# revision 18
# speedup vs baseline: 1.0330x; 1.0330x over previous
"""Trainium2 Bass kernel: 2-layer GCN (GCNConv -> ReLU -> GCNConv -> Linear).

Strategy (8 NeuronCores, SPMD), v3 "dense edge-slab" design:
  - Destination-node sharding; nodes assigned to (core, lane) by a
    degree-sorted serpentine so per-(core,tile) edge counts match across
    cores (minimal static padding).
  - 3 launches with host-side exchange of the small activation tables:
      A: H1 = X @ W1                      (row-sharded dense matmul)
      B: MP1 + b1 + ReLU, then @ (W2 Wp) -> T2   (feature-major out)
      C: MP2 + bpp -> y                   (feature-major out)
  - Message passing consumes a host-expanded *dense edge slab*: for each
    128-edge chunk the 128 source rows are laid out contiguously in DRAM
    (edge order, dest-sorted, self-loops included as ordinary edges).  The
    device streams the slab at full DMA bandwidth -- no dma_gather, no
    GpSimd descriptor generation (the baseline bottleneck).
  - Transposed MP matmul: out[f, lane] += slab_chunk[slot, f]^T-contracted
    with wsl[slot, lane-window].  The destination window lives in the PSUM
    *free* dim, so any [mn..mx] window is legal (single matmul per chunk).
    The first chunk of each tile stores a full 128-wide weight block and
    runs with start=True (PSUM zeroing without a bias bracket).
  - Biases ride the Scalar-engine activation (per-partition bias AP), not
    PE matmuls.  Outputs are staged in SBUF and written in 8-tile batches.
  - All matmul operands bf16 (fp32 PSUM accumulation); final output fp32
    (bf16 on the wire, upcast on host).
"""

from contextlib import ExitStack
from dataclasses import dataclass, field

import numpy as np
import ml_dtypes

BF16 = ml_dtypes.bfloat16
E4M3 = ml_dtypes.float8_e4m3fn
FP32 = np.float32


# ---------------------------------------------------------------- config

@dataclass
class Cfg:
    N: int = 50000
    IN_DIM: int = 512
    HID: int = 256
    OUT: int = 128
    NCORES: int = 8
    BLK_B: int = 32       # slab chunks per stream DMA, launch B (16KB/part)
    BLK_C: int = 64       # launch C (16KB/part)
    TB_A: int = 8         # x tiles per stream DMA, launch A (8KB/part)
    GRP: int = 8          # output tiles per batched store
    MASS_FRAC: float = 0.17   # fraction of sum(norm^2) carried by fp8 edges

    ND: int = field(init=False)
    NTILES: int = field(init=False)
    NP: int = field(init=False)

    def __post_init__(self):
        self.ND = self.N // self.NCORES
        self.NTILES = (self.ND + 127) // 128
        self.NP = self.NTILES * 128


# ---------------------------------------------------------------- planner

class Plan:
    """Static (cross-core identical) chunk geometry + per-core data."""

    def __init__(self, cfg: Cfg, edge_index, edge_weight):
        self.cfg = cfg
        N, ND, NP, NT = cfg.N, cfg.ND, cfg.NP, cfg.NTILES
        NC = cfg.NCORES

        # --- gcn_norm with self loops (kept as ordinary edges)
        row = np.concatenate([np.asarray(edge_index[0], np.int64),
                              np.arange(N, dtype=np.int64)])
        col = np.concatenate([np.asarray(edge_index[1], np.int64),
                              np.arange(N, dtype=np.int64)])
        w = np.concatenate([np.asarray(edge_weight, np.float64),
                            np.ones(N, np.float64)])
        deg = np.zeros(N, np.float64)
        np.add.at(deg, col, w)
        dinv = np.where(deg > 0, 1.0 / np.sqrt(deg), 0.0)
        nrm = (dinv[row] * w * dinv[col]).astype(np.float32)

        # --- degree-sorted serpentine node -> (core, lane)
        degi = np.bincount(col, minlength=N)
        ranks = np.argsort(-degi, kind="stable")    # rank r -> node
        r = np.arange(N)
        blk = r // NC
        corepos = np.where(blk % 2 == 0, r % NC, NC - 1 - (r % NC))
        lane_global = np.empty(N, np.int64)         # node -> core*NP + lane
        lane_global[ranks] = corepos * NP + blk
        self.nodes = []                             # per core: lane -> node id
        for k in range(NC):
            nk = np.empty(ND, np.int64)
            sel = corepos == k
            nk[blk[sel]] = ranks[sel]
            self.nodes.append(nk)

        # --- edge geometry, dest-sorted
        src_row = lane_global[row]                  # table row of the source
        dst = lane_global[col]
        dst_core = dst // NP
        dlane = dst % NP
        dtile = dlane // 128
        dl = dlane % 128

        order = np.lexsort((dl, dtile, dst_core))
        so_core = dst_core[order]
        so_tile = dtile[order]
        so_lane = dl[order]
        so_src = src_row[order]
        so_w = nrm[order]

        # --- fp8 / bf16 split: the low-|norm| edges carrying MASS_FRAC of
        # sum(norm^2) ride in fp8 slabs (per-source-row scaling folded into
        # their weight slab).  Self-loops stay bf16 so every (core,tile) has
        # a bf16 first chunk.
        so_self = np.concatenate([row[:len(row) - N] == col[:len(row) - N],
                                  np.ones(N, bool)])[order]
        w2 = so_w.astype(np.float64) ** 2
        ow = np.argsort(np.abs(so_w), kind="stable")
        cmass = np.cumsum(w2[ow])
        nf8 = int(np.searchsorted(cmass / max(cmass[-1], 1e-30),
                                  cfg.MASS_FRAC))
        f8 = np.zeros(len(so_w), bool)
        f8[ow[:nf8]] = True
        f8 &= ~self._selfmask(row, col, N, order)

        def geom(sel, force_first_full):
            seg_s = so_core[sel] * NT + so_tile[sel]
            cnt_s = np.bincount(seg_s, minlength=NC * NT).reshape(NC, NT)
            CH = (-(-cnt_s // 128)).max(axis=0)
            base = np.concatenate([[0], np.cumsum(CH)])
            TOT = int(base[-1])
            seg_start = np.concatenate(
                [[0], np.cumsum(np.bincount(seg_s, minlength=NC * NT))])[:-1]
            rank = np.arange(sel.sum()) - seg_start[seg_s]
            chunk = base[so_tile[sel]] + rank // 128
            slot = rank % 128
            mn = np.full(max(TOT, 1), 128, np.int64)
            mx = np.full(max(TOT, 1), -1, np.int64)
            lanes = so_lane[sel]
            np.minimum.at(mn, chunk, lanes)
            np.maximum.at(mx, chunk, lanes)
            empty = mx < 0
            mn[empty] = 0
            mx[empty] = mn[empty] - 1
            if force_first_full and TOT:
                first = base[:-1]
                mn[first] = 0
                mx[first] = 127
            span = mx - mn + 1
            off = np.concatenate([[0], np.cumsum(span)])
            return dict(CH=CH, base=base, TOT=TOT, chunk=chunk, slot=slot,
                        mn=mn, span=span, off=off,
                        SLAB=max(int(off[-1]), 1))

        gb = geom(~f8, True)
        g8 = geom(f8, False)
        self.gb, self.g8 = gb, g8
        self.TOTCH, self.SLAB = gb["TOT"], gb["SLAB"]
        self.TOTCH8, self.SLAB8 = max(g8["TOT"], 1), g8["SLAB"]

        # --- per-core arrays
        self.wsl = []       # bf16 one-hot*norm blocks [128, SLAB]
        self.pos = []       # bf16 edge -> slab position
        self.srcrow = []
        self.pos8 = []      # fp8 edge -> slab8 position
        self.srcrow8 = []
        self.w8 = []        # fp8 edge norms
        self.wcol8 = []     # fp8 edge -> (slot, wsl8 column)
        kb = so_core[~f8]
        k8 = so_core[f8]
        srcb, src8 = so_src[~f8], so_src[f8]
        laneb, lane8 = so_lane[~f8], so_lane[f8]
        wb, w8v = so_w[~f8], so_w[f8]
        for k in range(NC):
            m = kb == k
            kchunk, kslot = gb["chunk"][m], gb["slot"][m]
            wsl = np.zeros((128, self.SLAB), np.float32)
            wsl[kslot, gb["off"][kchunk] + (laneb[m] - gb["mn"][kchunk])] = wb[m]
            self.wsl.append(wsl.astype(BF16))
            self.pos.append(kslot * self.TOTCH + kchunk)
            self.srcrow.append(srcb[m])

            m8 = k8 == k
            kchunk8, kslot8 = g8["chunk"][m8], g8["slot"][m8]
            self.pos8.append(kslot8 * self.TOTCH8 + kchunk8)
            self.srcrow8.append(src8[m8])
            self.w8.append(w8v[m8].astype(np.float64))
            self.wcol8.append((kslot8,
                               g8["off"][kchunk8]
                               + (lane8[m8] - g8["mn"][kchunk8])))

    @staticmethod
    def _selfmask(row, col, N, order):
        m = np.zeros(len(row), bool)
        m[len(row) - N:] = True
        return m[order]

    @staticmethod
    def row_scales(tab: np.ndarray) -> np.ndarray:
        mx = np.maximum(np.abs(tab.astype(np.float32)).max(axis=1), 1e-20)
        return (240.0 / mx).astype(np.float32)

    def build_slab(self, k: int, tab: np.ndarray) -> np.ndarray:
        """Dense edge-ordered bf16 slab [128, TOTCH*F] for core k."""
        F = tab.shape[1]
        flat = np.zeros((128 * self.TOTCH, F), BF16)
        flat[self.pos[k]] = tab[self.srcrow[k]]
        return flat.reshape(128, self.TOTCH * F)

    def build_slab8(self, k: int, tab: np.ndarray, s: np.ndarray):
        """Dense edge-ordered fp8 slab [128, TOTCH8*F] (rows scaled by s)."""
        F = tab.shape[1]
        flat = np.zeros((128 * self.TOTCH8, F), E4M3)
        src = self.srcrow8[k]
        flat[self.pos8[k]] = (tab[src].astype(np.float32)
                              * s[src][:, None]).astype(E4M3)
        return flat.reshape(128, self.TOTCH8 * F)

    def build_wsl8(self, k: int, s: np.ndarray) -> np.ndarray:
        arr = np.zeros((128, self.SLAB8), np.float32)
        kslot, kcol = self.wcol8[k]
        arr[kslot, kcol] = self.w8[k] / s[self.srcrow8[k]]
        return arr.astype(BF16)


# ---------------------------------------------------------------- bass builders

def _build_l1(cfg: Cfg):
    """H1 = X @ W1, feature-major output (two halves h1a/h1b [128, NP]).
    xt lives fully in SBUF (12.5KB/part); few, large DMAs -- the SP
    sequencer's ~0.6us per dma_start was the previous bottleneck."""
    import concourse.bacc as bacc
    import concourse.mybir as mybir
    import concourse.tile as tile

    dt = mybir.dt
    nc = bacc.Bacc(None, target_bir_lowering=False, num_swdge_queues=4)
    KCH = cfg.IN_DIM // 128
    G = 4                                   # tiles per matmul (512 lanes)
    NG = -(-cfg.NTILES // G)
    L = G * 128
    SG = 4                                  # matmul groups per output stage
    # c-major x^T: block c is x[:, c*128:(c+1)*128].T laid out [128, NP]
    xt = nc.dram_tensor("xt", [128, KCH * cfg.NP], dt.bfloat16,
                        kind="ExternalInput")
    w1 = nc.dram_tensor("w1", [128, KCH * cfg.HID], dt.bfloat16,
                        kind="ExternalInput")
    outs_d = [nc.dram_tensor(f"h1{h}", [128, cfg.NP], dt.bfloat16,
                             kind="ExternalOutput") for h in range(2)]

    with tile.TileContext(nc) as tc, ExitStack() as ctx:
        consts = ctx.enter_context(tc.tile_pool(name="consts", bufs=1))
        stg = ctx.enter_context(tc.tile_pool(name="stg", bufs=2))
        pools = [ctx.enter_context(tc.tile_pool(name=f"psl{h}", bufs=3,
                                                space="PSUM"))
                 for h in range(2)]

        w1_sb = consts.tile([128, KCH * cfg.HID], dt.bfloat16, tag="w1")
        xt_sb = consts.tile([128, KCH * cfg.NP], dt.bfloat16, tag="xt")
        # interleave c-quarters so early (g, all-c) windows complete fast
        Q = 4
        qs = cfg.NP // Q
        nc.sync.dma_start(w1_sb[:], w1[:])
        for q in range(Q):
            for c in range(KCH):
                nc.sync.dma_start(
                    xt_sb[:, c * cfg.NP + q * qs: c * cfg.NP + (q + 1) * qs],
                    xt[:, c * cfg.NP + q * qs: c * cfg.NP + (q + 1) * qs])

        stages = [None, None]
        for g in range(NG):
            l0 = g * L
            l1 = min(cfg.NP, l0 + L)
            ll = l1 - l0
            sgi = g % SG
            if sgi == 0:
                nst = min(SG * L, cfg.NP - g * L)
                stages = [stg.tile([128, nst], dt.bfloat16, name="ostg")
                          for _ in range(2)]
            for h in range(2):
                ps = pools[h].tile([128, L], dt.float32, name="psl")
                for c in range(KCH):
                    nc.tensor.matmul(
                        ps[:, :ll],
                        w1_sb[:, c * cfg.HID + h * 128:
                              c * cfg.HID + (h + 1) * 128],
                        xt_sb[:, c * cfg.NP + l0: c * cfg.NP + l1],
                        start=(c == 0), stop=(c == KCH - 1),
                    )
                if h == 0:
                    nc.scalar.activation(
                        stages[h][:, sgi * L: sgi * L + ll], ps[:, :ll],
                        mybir.ActivationFunctionType.Copy)
                else:
                    nc.vector.tensor_copy(
                        stages[h][:, sgi * L: sgi * L + ll], ps[:, :ll])
            if sgi == SG - 1 or g == NG - 1:
                g0 = (g // SG) * SG * L
                for h in range(2):
                    nc.sync.dma_start(outs_d[h][:, g0:l1],
                                      stages[h][:, : l1 - g0])
    nc.finalize()
    return nc


def _build_mp(cfg: Cfg, plan: Plan, layer2: bool):
    """Transposed-MP launch.
    layer2: MP1 + b1 + ReLU + @W2p -> T2 [128, NP] bf16 (feature-major).
    else:   MP2 + bpp -> y [128, NP] bf16 (feature-major)."""
    import concourse.bacc as bacc
    import concourse.mybir as mybir
    import concourse.tile as tile

    dt = mybir.dt
    F = cfg.HID if layer2 else cfg.OUT          # slab feature width
    FCH = F // 128                              # psum column-tiles (2 or 1)
    BLK = cfg.BLK_B if layer2 else cfg.BLK_C
    BLK8 = 2 * BLK            # fp8 chunks are half the bytes
    GRP = cfg.GRP
    nc = bacc.Bacc(None, target_bir_lowering=False, num_swdge_queues=4)

    slab = nc.dram_tensor("slab", [128, plan.TOTCH * F], dt.bfloat16,
                          kind="ExternalInput")
    slab8 = nc.dram_tensor("slab8", [128, plan.TOTCH8 * F], dt.float8e4,
                           kind="ExternalInput")
    wsl = nc.dram_tensor("wsl", [128, plan.SLAB], dt.bfloat16,
                         kind="ExternalInput")
    wsl8 = nc.dram_tensor("wsl8", [128, plan.SLAB8], dt.bfloat16,
                          kind="ExternalInput")
    bvec = nc.dram_tensor("bvec", [128, FCH], dt.float32,
                          kind="ExternalInput")
    if layer2:
        w2p = nc.dram_tensor("w2p", [128, FCH * cfg.OUT], dt.bfloat16,
                             kind="ExternalInput")
    out = nc.dram_tensor("out", [128, cfg.NTILES * 128], dt.bfloat16,
                         kind="ExternalOutput")

    # split the wsl load at tile boundaries so early tiles' matmuls don't
    # wait on the whole slab-weight transfer
    nsplit = 4
    wcuts = [0]
    for i in range(1, nsplit):
        t = (cfg.NTILES * i) // nsplit
        wcuts.append(int(plan.gb["off"][plan.gb["base"][t]]))
    wcuts.append(plan.SLAB)

    with tile.TileContext(nc) as tc, ExitStack() as ctx:
        consts = ctx.enter_context(tc.tile_pool(name="consts", bufs=1))
        sstr = ctx.enter_context(tc.tile_pool(name="sstr", bufs=5))
        work = ctx.enter_context(tc.tile_pool(name="work", bufs=4))
        stg = ctx.enter_context(tc.tile_pool(name="stg", bufs=2))
        pools = [ctx.enter_context(tc.tile_pool(name=f"ps{c}",
                                                bufs=(3 if layer2 else 6),
                                                space="PSUM"))
                 for c in range(FCH)]
        if layer2:
            ps2p = ctx.enter_context(tc.tile_pool(name="ps2", bufs=2,
                                                  space="PSUM"))

        s8str = ctx.enter_context(tc.tile_pool(name="s8str", bufs=3))
        wsl_sb = consts.tile([128, plan.SLAB], dt.bfloat16, tag="wsl")
        wsl8_sb = consts.tile([128, plan.SLAB8], dt.bfloat16, tag="wsl8")
        bvec_sb = consts.tile([128, FCH], dt.float32, tag="bvec")
        if layer2:
            w2p_sb = consts.tile([128, FCH * cfg.OUT], dt.bfloat16, tag="w2p")

        stiles = {}

        def sblock(b):
            if b not in stiles:
                t = sstr.tile([128, BLK * F], dt.bfloat16, name="sb")
                c0 = b * BLK * F
                c1 = min(plan.TOTCH * F, c0 + BLK * F)
                nc.sync.dma_start(t[:, : c1 - c0], slab[:, c0:c1])
                stiles[b] = t
            return stiles[b]

        s8tiles = {}

        def s8block(b):
            if b not in s8tiles:
                t = s8str.tile([128, BLK8 * F], dt.float8e4, name="s8b")
                c0 = b * BLK8 * F
                c1 = min(plan.TOTCH8 * F, c0 + BLK8 * F)
                nc.sync.dma_start(t[:, : c1 - c0], slab8[:, c0:c1])
                s8tiles[b] = t
            return s8tiles[b]

        sblock(0)                 # slab block 0 first on the DMA queue
        for i in range(nsplit):
            nc.sync.dma_start(wsl_sb[:, wcuts[i]:wcuts[i + 1]],
                              wsl[:, wcuts[i]:wcuts[i + 1]])
        h8 = plan.SLAB8 // 2
        nc.sync.dma_start(wsl8_sb[:, :h8], wsl8[:, :h8])
        nc.sync.dma_start(wsl8_sb[:, h8:], wsl8[:, h8:])
        nc.sync.dma_start(bvec_sb[:], bvec[:])
        if layer2:
            nc.sync.dma_start(w2p_sb[:], w2p[:])

        stage = None
        for t in range(cfg.NTILES):
            g = t % GRP
            if g == 0:
                ntg = min(GRP, cfg.NTILES - t)
                stage = stg.tile([128, ntg * 128], dt.bfloat16,
                                 name="stage")
            pss = [pools[c].tile([128, 128], dt.float32, name=f"pst")
                   for c in range(FCH)]
            gb, g8 = plan.gb, plan.g8
            j0, j1 = int(gb["base"][t]), int(gb["base"][t + 1])
            e0, e1 = int(g8["base"][t]), int(g8["base"][t + 1])
            work_items = [(False, j) for j in range(j0, j1)
                          if gb["span"][j] > 0]
            work_items += [(True, j) for j in range(e0, e1)
                           if g8["span"][j] > 0]
            for wi, (is8, j) in enumerate(work_items):
                gg = g8 if is8 else gb
                sp = int(gg["span"][j])
                o0 = int(gg["off"][j])
                m0 = int(gg["mn"][j])
                if is8:
                    st = s8block(j // BLK8)
                    soff = (j % BLK8) * F
                    wtile = wsl8_sb
                else:
                    st = sblock(j // BLK)
                    soff = (j % BLK) * F
                    wtile = wsl_sb
                for c in range(FCH):
                    nc.tensor.matmul(
                        pss[c][:, m0:m0 + sp],
                        st[:, soff + c * 128: soff + (c + 1) * 128],
                        wtile[:, o0:o0 + sp],
                        start=(wi == 0), stop=(wi == len(work_items) - 1),
                        skip_group_check=True,
                    )

            if layer2:
                acts = []
                for c in range(FCH):
                    a = work.tile([128, 128], dt.bfloat16, name="act")
                    nc.scalar.activation(a[:], pss[c][:],
                                         mybir.ActivationFunctionType.Relu,
                                         bias=bvec_sb[:, c:c + 1])
                    acts.append(a)
                ps2 = ps2p.tile([128, cfg.OUT], dt.float32)
                for c in range(FCH):
                    nc.tensor.matmul(ps2[:],
                                     w2p_sb[:, c * cfg.OUT:(c + 1) * cfg.OUT],
                                     acts[c][:],
                                     start=(c == 0), stop=(c == FCH - 1))
                nc.scalar.activation(stage[:, g * 128:(g + 1) * 128], ps2[:],
                                     mybir.ActivationFunctionType.Copy)
            else:
                nc.scalar.add(stage[:, g * 128:(g + 1) * 128], pss[0][:],
                              bvec_sb[:, 0:1])

            if g == GRP - 1 or t == cfg.NTILES - 1:
                t0 = t - g
                nc.sync.dma_start(out[:, t0 * 128:(t + 1) * 128],
                                  stage[:, :(g + 1) * 128])

    nc.finalize()
    return nc


# ---------------------------------------------------------------- host packing

def _pack_l1_inputs(cfg: Cfg, plan: Plan, x, W1):
    KCH = cfg.IN_DIM // 128
    w1r = np.zeros((128, KCH * cfg.HID), BF16)
    for c in range(KCH):
        w1r[:, c * cfg.HID:(c + 1) * cfg.HID] = \
            W1[c * 128:(c + 1) * 128, :].astype(BF16)
    maps = []
    for k in range(cfg.NCORES):
        xs = np.zeros((cfg.NP, cfg.IN_DIM), np.float32)
        xs[:cfg.ND] = x[plan.nodes[k]]
        xtr = np.zeros((128, KCH * cfg.NP), BF16)
        for c in range(KCH):
            xtr[:, c * cfg.NP:(c + 1) * cfg.NP] = \
                xs[:, c * 128:(c + 1) * 128].T.astype(BF16)
        maps.append({"xt": xtr, "w1": w1r})
    return maps


def _pack_mp_inputs(cfg: Cfg, plan: Plan, table, Wn, b, layer2):
    F = cfg.HID if layer2 else cfg.OUT
    FCH = F // 128
    bvec = np.zeros((128, FCH), np.float32)
    for c in range(FCH):
        bvec[:, c] = b[c * 128:(c + 1) * 128]
    scales = Plan.row_scales(table)
    maps = []
    for k in range(cfg.NCORES):
        m = {
            "slab": plan.build_slab(k, table),
            "slab8": plan.build_slab8(k, table, scales),
            "wsl": plan.wsl[k],
            "wsl8": plan.build_wsl8(k, scales),
            "bvec": bvec,
        }
        if layer2:
            wnr = np.zeros((128, FCH * cfg.OUT), BF16)
            for c in range(FCH):
                wnr[:, c * cfg.OUT:(c + 1) * cfg.OUT] = \
                    Wn[c * 128:(c + 1) * 128, :].astype(BF16)
            m["w2p"] = wnr
        maps.append(m)
    return maps


# ---------------------------------------------------------------- driver

def _run(nc, in_maps, cfg, trace=False):
    from concourse.bass_utils import run_bass_kernel_spmd
    res = run_bass_kernel_spmd(nc, in_maps, list(range(cfg.NCORES)), trace=trace)
    return res


def kernel_run(inputs, cfg=None, trace=False, sim=False):
    cfg = cfg or Cfg()
    x = np.asarray(inputs["x"], np.float32)
    plan = Plan(cfg, np.asarray(inputs["edge_index"]),
                np.asarray(inputs["edge_weight"], np.float32))
    W1 = np.asarray(inputs["W1"], np.float32)
    b1 = np.asarray(inputs["b1"], np.float32)
    W2 = np.asarray(inputs["W2"], np.float32)
    b2 = np.asarray(inputs["b2"], np.float32)
    Wp = np.asarray(inputs["Wp"], np.float32)
    bp = np.asarray(inputs["bp"], np.float32)

    results = []

    def run(build, maps, outname):
        nc = build()
        if sim:
            from concourse.bass_interp import CoreSim
            outs = []
            for k in range(cfg.NCORES):
                s = CoreSim(nc)
                for name, arr in maps[k].items():
                    s.tensor(name)[:] = arr
                s.simulate()
                outs.append({outname: s.tensor(outname).copy()})
            results.append(None)
            return outs
        r = _run(nc, maps, cfg, trace=trace)
        results.append(r)
        return r.results

    # fold the post-projection into layer 2: A(relu1@W2)@Wp = A(relu1@(W2@Wp))
    W2p = (W2 @ Wp).astype(np.float32)
    bpp = (b2 @ Wp + bp).astype(np.float32)

    def as_bf16(a):
        a = np.asarray(a)
        return a if a.dtype == BF16 else a.view(BF16)

    r1 = run(lambda: _build_l1(cfg), _pack_l1_inputs(cfg, plan, x, W1), "h1")
    T1 = np.concatenate(
        [np.concatenate([as_bf16(r["h10"]).T, as_bf16(r["h11"]).T], axis=1)
         for r in r1], axis=0)

    r2 = run(lambda: _build_mp(cfg, plan, True),
             _pack_mp_inputs(cfg, plan, T1, W2p, b1, True), "out")
    # feature-major [128, NP] -> row-major table [NCORES*NP, 128]
    T2 = np.concatenate([as_bf16(r["out"]).T for r in r2], axis=0)

    r3 = run(lambda: _build_mp(cfg, plan, False),
             _pack_mp_inputs(cfg, plan, T2, None, bpp, False), "out")

    y = np.empty((cfg.N, cfg.OUT), np.float32)
    for k in range(cfg.NCORES):
        shard = as_bf16(r3[k]["out"]).T.astype(np.float32)   # [NP, OUT]
        y[plan.nodes[k]] = shard[:cfg.ND]
    return y, results


def kernel(**inputs):
    y, _ = kernel_run(inputs)
    return y


# revision 20
# speedup vs baseline: 1.1388x; 1.1025x over previous
"""Trainium2 Bass kernel: 2-layer GCN (GCNConv -> ReLU -> GCNConv -> Linear).

Strategy (8 NeuronCores, SPMD), v3 "dense edge-slab" design:
  - Destination-node sharding; nodes assigned to (core, lane) by a
    degree-sorted serpentine so per-(core,tile) edge counts match across
    cores (minimal static padding).
  - 3 launches with host-side exchange of the small activation tables:
      A: H1 = X @ W1                      (row-sharded dense matmul)
      B: MP1 + b1 + ReLU, then @ (W2 Wp) -> T2   (feature-major out)
      C: MP2 + bpp -> y                   (feature-major out)
  - Message passing consumes a host-expanded *dense edge slab*: for each
    128-edge chunk the 128 source rows are laid out contiguously in DRAM
    (edge order, dest-sorted, self-loops included as ordinary edges).  The
    device streams the slab at full DMA bandwidth -- no dma_gather, no
    GpSimd descriptor generation (the baseline bottleneck).
  - Transposed MP matmul: out[f, lane] += slab_chunk[slot, f]^T-contracted
    with wsl[slot, lane-window].  The destination window lives in the PSUM
    *free* dim, so any [mn..mx] window is legal (single matmul per chunk).
    The first chunk of each tile stores a full 128-wide weight block and
    runs with start=True (PSUM zeroing without a bias bracket).
  - Biases ride the Scalar-engine activation (per-partition bias AP), not
    PE matmuls.  Outputs are staged in SBUF and written in 8-tile batches.
  - All matmul operands bf16 (fp32 PSUM accumulation); final output fp32
    (bf16 on the wire, upcast on host).
"""

from contextlib import ExitStack
from dataclasses import dataclass, field

import numpy as np
import ml_dtypes

BF16 = ml_dtypes.bfloat16
E4M3 = ml_dtypes.float8_e4m3fn
FP32 = np.float32


# ---------------------------------------------------------------- config

@dataclass
class Cfg:
    N: int = 50000
    IN_DIM: int = 512
    HID: int = 256
    OUT: int = 128
    NCORES: int = 8
    BLK_B: int = 32       # slab chunks per stream DMA, launch B (16KB/part)
    BLK_C: int = 64       # launch C (16KB/part)
    TB_A: int = 8         # x tiles per stream DMA, launch A (8KB/part)
    GRP: int = 8          # output tiles per batched store
    F8_FRAC: float = 0.75     # target fraction of non-self edges in fp8

    ND: int = field(init=False)
    NTILES: int = field(init=False)
    NP: int = field(init=False)

    def __post_init__(self):
        self.ND = self.N // self.NCORES
        self.NTILES = (self.ND + 127) // 128
        self.NP = self.NTILES * 128


# ---------------------------------------------------------------- planner

class Plan:
    """Static (cross-core identical) chunk geometry + per-core data."""

    def __init__(self, cfg: Cfg, edge_index, edge_weight):
        self.cfg = cfg
        N, ND, NP, NT = cfg.N, cfg.ND, cfg.NP, cfg.NTILES
        NC = cfg.NCORES

        # --- gcn_norm with self loops (kept as ordinary edges)
        row = np.concatenate([np.asarray(edge_index[0], np.int64),
                              np.arange(N, dtype=np.int64)])
        col = np.concatenate([np.asarray(edge_index[1], np.int64),
                              np.arange(N, dtype=np.int64)])
        w = np.concatenate([np.asarray(edge_weight, np.float64),
                            np.ones(N, np.float64)])
        deg = np.zeros(N, np.float64)
        np.add.at(deg, col, w)
        dinv = np.where(deg > 0, 1.0 / np.sqrt(deg), 0.0)
        nrm = (dinv[row] * w * dinv[col]).astype(np.float32)

        # --- degree-sorted serpentine node -> (core, lane)
        degi = np.bincount(col, minlength=N)
        ranks = np.argsort(-degi, kind="stable")    # rank r -> node
        r = np.arange(N)
        blk = r // NC
        corepos = np.where(blk % 2 == 0, r % NC, NC - 1 - (r % NC))
        lane_global = np.empty(N, np.int64)         # node -> core*NP + lane
        lane_global[ranks] = corepos * NP + blk
        self.nodes = []                             # per core: lane -> node id
        for k in range(NC):
            nk = np.empty(ND, np.int64)
            sel = corepos == k
            nk[blk[sel]] = ranks[sel]
            self.nodes.append(nk)

        # --- edge geometry, dest-sorted
        src_row = lane_global[row]                  # table row of the source
        dst = lane_global[col]
        dst_core = dst // NP
        dlane = dst % NP
        dtile = dlane // 128
        dl = dlane % 128

        order = np.lexsort((dl, dtile, dst_core))
        so_core = dst_core[order]
        so_tile = dtile[order]
        so_lane = dl[order]
        so_src = src_row[order]
        so_w = nrm[order]

        # --- fp8 / bf16 split, chunk-aligned per (core,tile): every core
        # contributes exactly C8[t]*128 of its locally-lowest-|norm| non-self
        # edges to the fp8 stream, so the fp8 slab has zero padding.
        # Self-loops stay bf16 so every (core,tile) keeps a bf16 first chunk.
        so_self = self._selfmask(row, col, N, order)
        seg0 = so_core * NT + so_tile
        cnt0 = np.bincount(seg0, minlength=NC * NT).reshape(NC, NT)
        avail = cnt0 - 128                     # non-self edges per (core,tile)
        tgt = np.round(cfg.F8_FRAC * avail.mean(axis=0) / 128).astype(np.int64)
        C8t = np.minimum(tgt, (avail // 128).min(axis=0))
        C8t = np.maximum(C8t, 0)

        ordx = np.lexsort((np.abs(so_w), so_self, seg0))
        segstart = np.concatenate(
            [[0], np.cumsum(np.bincount(seg0, minlength=NC * NT))])[:-1]
        rank_in = np.arange(len(ordx)) - segstart[seg0[ordx]]
        take = rank_in < (C8t[so_tile[ordx]] * 128)
        f8 = np.zeros(len(so_w), bool)
        f8[ordx[take]] = True
        f8 &= ~so_self

        def geom(sel, force_first_full):
            seg_s = so_core[sel] * NT + so_tile[sel]
            cnt_s = np.bincount(seg_s, minlength=NC * NT).reshape(NC, NT)
            CH = (-(-cnt_s // 128)).max(axis=0)
            base = np.concatenate([[0], np.cumsum(CH)])
            TOT = int(base[-1])
            seg_start = np.concatenate(
                [[0], np.cumsum(np.bincount(seg_s, minlength=NC * NT))])[:-1]
            rank = np.arange(sel.sum()) - seg_start[seg_s]
            chunk = base[so_tile[sel]] + rank // 128
            slot = rank % 128
            mn = np.full(max(TOT, 1), 128, np.int64)
            mx = np.full(max(TOT, 1), -1, np.int64)
            lanes = so_lane[sel]
            np.minimum.at(mn, chunk, lanes)
            np.maximum.at(mx, chunk, lanes)
            empty = mx < 0
            mn[empty] = 0
            mx[empty] = mn[empty] - 1
            if force_first_full and TOT:
                first = base[:-1]
                mn[first] = 0
                mx[first] = 127
            span = mx - mn + 1
            off = np.concatenate([[0], np.cumsum(span)])
            return dict(CH=CH, base=base, TOT=TOT, chunk=chunk, slot=slot,
                        mn=mn, span=span, off=off,
                        SLAB=max(int(off[-1]), 1))

        gb = geom(~f8, True)
        g8 = geom(f8, False)
        self.gb, self.g8 = gb, g8
        self.TOTCH, self.SLAB = gb["TOT"], gb["SLAB"]
        self.TOTCH8, self.SLAB8 = max(g8["TOT"], 1), g8["SLAB"]

        # --- per-core arrays
        self.wsl = []       # bf16 one-hot*norm blocks [128, SLAB]
        self.pos = []       # bf16 edge -> slab position
        self.srcrow = []
        self.pos8 = []      # fp8 edge -> slab8 position
        self.srcrow8 = []
        self.w8 = []        # fp8 edge norms
        self.wcol8 = []     # fp8 edge -> (slot, wsl8 column)
        kb = so_core[~f8]
        k8 = so_core[f8]
        srcb, src8 = so_src[~f8], so_src[f8]
        laneb, lane8 = so_lane[~f8], so_lane[f8]
        wb, w8v = so_w[~f8], so_w[f8]
        for k in range(NC):
            m = kb == k
            kchunk, kslot = gb["chunk"][m], gb["slot"][m]
            wsl = np.zeros((128, self.SLAB), np.float32)
            wsl[kslot, gb["off"][kchunk] + (laneb[m] - gb["mn"][kchunk])] = wb[m]
            self.wsl.append(wsl.astype(BF16))
            self.pos.append(kslot * self.TOTCH + kchunk)
            self.srcrow.append(srcb[m])

            m8 = k8 == k
            kchunk8, kslot8 = g8["chunk"][m8], g8["slot"][m8]
            self.pos8.append(kslot8 * self.TOTCH8 + kchunk8)
            self.srcrow8.append(src8[m8])
            self.w8.append(w8v[m8].astype(np.float64))
            self.wcol8.append((kslot8,
                               g8["off"][kchunk8]
                               + (lane8[m8] - g8["mn"][kchunk8])))

    @staticmethod
    def _selfmask(row, col, N, order):
        m = np.zeros(len(row), bool)
        m[len(row) - N:] = True
        return m[order]

    @staticmethod
    def row_scales(tab: np.ndarray) -> np.ndarray:
        mx = np.maximum(np.abs(tab.astype(np.float32)).max(axis=1), 1e-20)
        return (240.0 / mx).astype(np.float32)

    def build_slab(self, k: int, tab: np.ndarray) -> np.ndarray:
        """Dense edge-ordered bf16 slab [128, TOTCH*F] for core k."""
        F = tab.shape[1]
        flat = np.zeros((128 * self.TOTCH, F), BF16)
        flat[self.pos[k]] = tab[self.srcrow[k]]
        return flat.reshape(128, self.TOTCH * F)

    def build_slab8(self, k: int, tab: np.ndarray, s: np.ndarray):
        """Dense edge-ordered fp8 slab [128, TOTCH8*F] (rows scaled by s)."""
        F = tab.shape[1]
        flat = np.zeros((128 * self.TOTCH8, F), E4M3)
        src = self.srcrow8[k]
        flat[self.pos8[k]] = (tab[src].astype(np.float32)
                              * s[src][:, None]).astype(E4M3)
        return flat.reshape(128, self.TOTCH8 * F)

    def build_wsl8(self, k: int, s: np.ndarray) -> np.ndarray:
        arr = np.zeros((128, self.SLAB8), np.float32)
        kslot, kcol = self.wcol8[k]
        arr[kslot, kcol] = self.w8[k] / s[self.srcrow8[k]]
        return arr.astype(BF16)


# ---------------------------------------------------------------- bass builders

def _build_l1(cfg: Cfg):
    """H1 = X @ W1, feature-major output (two halves h1a/h1b [128, NP]).
    xt lives fully in SBUF (12.5KB/part); few, large DMAs -- the SP
    sequencer's ~0.6us per dma_start was the previous bottleneck."""
    import concourse.bacc as bacc
    import concourse.mybir as mybir
    import concourse.tile as tile

    dt = mybir.dt
    nc = bacc.Bacc(None, target_bir_lowering=False, num_swdge_queues=4)
    KCH = cfg.IN_DIM // 128
    G = 4                                   # tiles per matmul (512 lanes)
    NG = -(-cfg.NTILES // G)
    L = G * 128
    SG = 4                                  # matmul groups per output stage
    # c-major x^T: block c is x[:, c*128:(c+1)*128].T laid out [128, NP]
    xt = nc.dram_tensor("xt", [128, KCH * cfg.NP], dt.bfloat16,
                        kind="ExternalInput")
    w1 = nc.dram_tensor("w1", [128, KCH * cfg.HID], dt.bfloat16,
                        kind="ExternalInput")
    outs_d = [nc.dram_tensor(f"h1{h}", [128, cfg.NP], dt.bfloat16,
                             kind="ExternalOutput") for h in range(2)]

    with tile.TileContext(nc) as tc, ExitStack() as ctx:
        consts = ctx.enter_context(tc.tile_pool(name="consts", bufs=1))
        stg = ctx.enter_context(tc.tile_pool(name="stg", bufs=2))
        pools = [ctx.enter_context(tc.tile_pool(name=f"psl{h}", bufs=3,
                                                space="PSUM"))
                 for h in range(2)]

        w1_sb = consts.tile([128, KCH * cfg.HID], dt.bfloat16, tag="w1")
        xt_sb = consts.tile([128, KCH * cfg.NP], dt.bfloat16, tag="xt")
        # interleave c-quarters so early (g, all-c) windows complete fast
        Q = 4
        qs = cfg.NP // Q
        nc.sync.dma_start(w1_sb[:], w1[:])
        for q in range(Q):
            for c in range(KCH):
                nc.sync.dma_start(
                    xt_sb[:, c * cfg.NP + q * qs: c * cfg.NP + (q + 1) * qs],
                    xt[:, c * cfg.NP + q * qs: c * cfg.NP + (q + 1) * qs])

        stages = [None, None]
        for g in range(NG):
            l0 = g * L
            l1 = min(cfg.NP, l0 + L)
            ll = l1 - l0
            sgi = g % SG
            if sgi == 0:
                nst = min(SG * L, cfg.NP - g * L)
                stages = [stg.tile([128, nst], dt.bfloat16, name="ostg")
                          for _ in range(2)]
            for h in range(2):
                ps = pools[h].tile([128, L], dt.float32, name="psl")
                for c in range(KCH):
                    nc.tensor.matmul(
                        ps[:, :ll],
                        w1_sb[:, c * cfg.HID + h * 128:
                              c * cfg.HID + (h + 1) * 128],
                        xt_sb[:, c * cfg.NP + l0: c * cfg.NP + l1],
                        start=(c == 0), stop=(c == KCH - 1),
                    )
                if h == 0:
                    nc.scalar.activation(
                        stages[h][:, sgi * L: sgi * L + ll], ps[:, :ll],
                        mybir.ActivationFunctionType.Copy)
                else:
                    nc.vector.tensor_copy(
                        stages[h][:, sgi * L: sgi * L + ll], ps[:, :ll])
            if sgi == SG - 1 or g == NG - 1:
                g0 = (g // SG) * SG * L
                for h in range(2):
                    nc.sync.dma_start(outs_d[h][:, g0:l1],
                                      stages[h][:, : l1 - g0])
    nc.finalize()
    return nc


def _build_mp(cfg: Cfg, plan: Plan, layer2: bool):
    """Transposed-MP launch.
    layer2: MP1 + b1 + ReLU + @W2p -> T2 [128, NP] bf16 (feature-major).
    else:   MP2 + bpp -> y [128, NP] bf16 (feature-major)."""
    import concourse.bacc as bacc
    import concourse.mybir as mybir
    import concourse.tile as tile

    dt = mybir.dt
    F = cfg.HID if layer2 else cfg.OUT          # slab feature width
    FCH = F // 128                              # psum column-tiles (2 or 1)
    BLK = cfg.BLK_B if layer2 else cfg.BLK_C
    BLK8 = 2 * BLK            # fp8 chunks are half the bytes
    GRP = cfg.GRP
    nc = bacc.Bacc(None, target_bir_lowering=False, num_swdge_queues=4)

    slab = nc.dram_tensor("slab", [128, plan.TOTCH * F], dt.bfloat16,
                          kind="ExternalInput")
    slab8 = nc.dram_tensor("slab8", [128, plan.TOTCH8 * F], dt.float8e4,
                           kind="ExternalInput")
    wsl = nc.dram_tensor("wsl", [128, plan.SLAB], dt.bfloat16,
                         kind="ExternalInput")
    wsl8 = nc.dram_tensor("wsl8", [128, plan.SLAB8], dt.bfloat16,
                          kind="ExternalInput")
    bvec = nc.dram_tensor("bvec", [128, FCH], dt.float32,
                          kind="ExternalInput")
    if layer2:
        w2p = nc.dram_tensor("w2p", [128, FCH * cfg.OUT], dt.bfloat16,
                             kind="ExternalInput")
    out = nc.dram_tensor("out", [128, cfg.NTILES * 128], dt.bfloat16,
                         kind="ExternalOutput")

    # split the wsl load at tile boundaries so early tiles' matmuls don't
    # wait on the whole slab-weight transfer
    nsplit = 4
    wcuts = [0]
    for i in range(1, nsplit):
        t = (cfg.NTILES * i) // nsplit
        wcuts.append(int(plan.gb["off"][plan.gb["base"][t]]))
    wcuts.append(plan.SLAB)

    with tile.TileContext(nc) as tc, ExitStack() as ctx:
        consts = ctx.enter_context(tc.tile_pool(name="consts", bufs=1))
        sstr = ctx.enter_context(tc.tile_pool(name="sstr", bufs=5))
        work = ctx.enter_context(tc.tile_pool(name="work", bufs=4))
        stg = ctx.enter_context(tc.tile_pool(name="stg", bufs=2))
        pools = [ctx.enter_context(tc.tile_pool(name=f"ps{c}",
                                                bufs=(3 if layer2 else 6),
                                                space="PSUM"))
                 for c in range(FCH)]
        if layer2:
            ps2p = ctx.enter_context(tc.tile_pool(name="ps2", bufs=2,
                                                  space="PSUM"))

        s8str = ctx.enter_context(tc.tile_pool(name="s8str", bufs=3))
        wsl_sb = consts.tile([128, plan.SLAB], dt.bfloat16, tag="wsl")
        wsl8_sb = consts.tile([128, plan.SLAB8], dt.bfloat16, tag="wsl8")
        bvec_sb = consts.tile([128, FCH], dt.float32, tag="bvec")
        if layer2:
            w2p_sb = consts.tile([128, FCH * cfg.OUT], dt.bfloat16, tag="w2p")

        stiles = {}

        def sblock(b):
            if b not in stiles:
                t = sstr.tile([128, BLK * F], dt.bfloat16, name="sb")
                c0 = b * BLK * F
                c1 = min(plan.TOTCH * F, c0 + BLK * F)
                nc.sync.dma_start(t[:, : c1 - c0], slab[:, c0:c1])
                stiles[b] = t
            return stiles[b]

        s8tiles = {}

        def s8block(b):
            if b not in s8tiles:
                t = s8str.tile([128, BLK8 * F], dt.float8e4, name="s8b")
                c0 = b * BLK8 * F
                c1 = min(plan.TOTCH8 * F, c0 + BLK8 * F)
                nc.sync.dma_start(t[:, : c1 - c0], slab8[:, c0:c1])
                s8tiles[b] = t
            return s8tiles[b]

        sblock(0)                 # slab block 0 first on the DMA queue
        for i in range(nsplit):
            nc.sync.dma_start(wsl_sb[:, wcuts[i]:wcuts[i + 1]],
                              wsl[:, wcuts[i]:wcuts[i + 1]])
        h8 = plan.SLAB8 // 2
        nc.sync.dma_start(wsl8_sb[:, :h8], wsl8[:, :h8])
        nc.sync.dma_start(wsl8_sb[:, h8:], wsl8[:, h8:])
        nc.sync.dma_start(bvec_sb[:], bvec[:])
        if layer2:
            nc.sync.dma_start(w2p_sb[:], w2p[:])

        stage = None
        for t in range(cfg.NTILES):
            g = t % GRP
            if g == 0:
                ntg = min(GRP, cfg.NTILES - t)
                stage = stg.tile([128, ntg * 128], dt.bfloat16,
                                 name="stage")
            pss = [pools[c].tile([128, 128], dt.float32, name=f"pst")
                   for c in range(FCH)]
            gb, g8 = plan.gb, plan.g8
            j0, j1 = int(gb["base"][t]), int(gb["base"][t + 1])
            e0, e1 = int(g8["base"][t]), int(g8["base"][t + 1])
            work_items = [(False, j) for j in range(j0, j1)
                          if gb["span"][j] > 0]
            work_items += [(True, j) for j in range(e0, e1)
                           if g8["span"][j] > 0]
            for wi, (is8, j) in enumerate(work_items):
                gg = g8 if is8 else gb
                sp = int(gg["span"][j])
                o0 = int(gg["off"][j])
                m0 = int(gg["mn"][j])
                if is8:
                    st = s8block(j // BLK8)
                    soff = (j % BLK8) * F
                    wtile = wsl8_sb
                else:
                    st = sblock(j // BLK)
                    soff = (j % BLK) * F
                    wtile = wsl_sb
                for c in range(FCH):
                    nc.tensor.matmul(
                        pss[c][:, m0:m0 + sp],
                        st[:, soff + c * 128: soff + (c + 1) * 128],
                        wtile[:, o0:o0 + sp],
                        start=(wi == 0), stop=(wi == len(work_items) - 1),
                        skip_group_check=True,
                    )

            if layer2:
                acts = []
                for c in range(FCH):
                    a = work.tile([128, 128], dt.bfloat16, name="act")
                    nc.scalar.activation(a[:], pss[c][:],
                                         mybir.ActivationFunctionType.Relu,
                                         bias=bvec_sb[:, c:c + 1])
                    acts.append(a)
                ps2 = ps2p.tile([128, cfg.OUT], dt.float32)
                for c in range(FCH):
                    nc.tensor.matmul(ps2[:],
                                     w2p_sb[:, c * cfg.OUT:(c + 1) * cfg.OUT],
                                     acts[c][:],
                                     start=(c == 0), stop=(c == FCH - 1))
                nc.scalar.activation(stage[:, g * 128:(g + 1) * 128], ps2[:],
                                     mybir.ActivationFunctionType.Copy)
            else:
                nc.scalar.add(stage[:, g * 128:(g + 1) * 128], pss[0][:],
                              bvec_sb[:, 0:1])

            if g == GRP - 1 or t == cfg.NTILES - 1:
                t0 = t - g
                nc.sync.dma_start(out[:, t0 * 128:(t + 1) * 128],
                                  stage[:, :(g + 1) * 128])

    nc.finalize()
    return nc


# ---------------------------------------------------------------- host packing

def _pack_l1_inputs(cfg: Cfg, plan: Plan, x, W1):
    KCH = cfg.IN_DIM // 128
    w1r = np.zeros((128, KCH * cfg.HID), BF16)
    for c in range(KCH):
        w1r[:, c * cfg.HID:(c + 1) * cfg.HID] = \
            W1[c * 128:(c + 1) * 128, :].astype(BF16)
    maps = []
    for k in range(cfg.NCORES):
        xs = np.zeros((cfg.NP, cfg.IN_DIM), np.float32)
        xs[:cfg.ND] = x[plan.nodes[k]]
        xtr = np.zeros((128, KCH * cfg.NP), BF16)
        for c in range(KCH):
            xtr[:, c * cfg.NP:(c + 1) * cfg.NP] = \
                xs[:, c * 128:(c + 1) * 128].T.astype(BF16)
        maps.append({"xt": xtr, "w1": w1r})
    return maps


def _pack_mp_inputs(cfg: Cfg, plan: Plan, table, Wn, b, layer2):
    F = cfg.HID if layer2 else cfg.OUT
    FCH = F // 128
    bvec = np.zeros((128, FCH), np.float32)
    for c in range(FCH):
        bvec[:, c] = b[c * 128:(c + 1) * 128]
    scales = Plan.row_scales(table)
    maps = []
    for k in range(cfg.NCORES):
        m = {
            "slab": plan.build_slab(k, table),
            "slab8": plan.build_slab8(k, table, scales),
            "wsl": plan.wsl[k],
            "wsl8": plan.build_wsl8(k, scales),
            "bvec": bvec,
        }
        if layer2:
            wnr = np.zeros((128, FCH * cfg.OUT), BF16)
            for c in range(FCH):
                wnr[:, c * cfg.OUT:(c + 1) * cfg.OUT] = \
                    Wn[c * 128:(c + 1) * 128, :].astype(BF16)
            m["w2p"] = wnr
        maps.append(m)
    return maps


# ---------------------------------------------------------------- driver

def _run(nc, in_maps, cfg, trace=False):
    from concourse.bass_utils import run_bass_kernel_spmd
    res = run_bass_kernel_spmd(nc, in_maps, list(range(cfg.NCORES)), trace=trace)
    return res


def kernel_run(inputs, cfg=None, trace=False, sim=False):
    cfg = cfg or Cfg()
    x = np.asarray(inputs["x"], np.float32)
    plan = Plan(cfg, np.asarray(inputs["edge_index"]),
                np.asarray(inputs["edge_weight"], np.float32))
    W1 = np.asarray(inputs["W1"], np.float32)
    b1 = np.asarray(inputs["b1"], np.float32)
    W2 = np.asarray(inputs["W2"], np.float32)
    b2 = np.asarray(inputs["b2"], np.float32)
    Wp = np.asarray(inputs["Wp"], np.float32)
    bp = np.asarray(inputs["bp"], np.float32)

    results = []

    def run(build, maps, outname):
        nc = build()
        if sim:
            from concourse.bass_interp import CoreSim
            outs = []
            for k in range(cfg.NCORES):
                s = CoreSim(nc)
                for name, arr in maps[k].items():
                    s.tensor(name)[:] = arr
                s.simulate()
                outs.append({outname: s.tensor(outname).copy()})
            results.append(None)
            return outs
        r = _run(nc, maps, cfg, trace=trace)
        results.append(r)
        return r.results

    # fold the post-projection into layer 2: A(relu1@W2)@Wp = A(relu1@(W2@Wp))
    W2p = (W2 @ Wp).astype(np.float32)
    bpp = (b2 @ Wp + bp).astype(np.float32)

    def as_bf16(a):
        a = np.asarray(a)
        return a if a.dtype == BF16 else a.view(BF16)

    r1 = run(lambda: _build_l1(cfg), _pack_l1_inputs(cfg, plan, x, W1), "h1")
    T1 = np.concatenate(
        [np.concatenate([as_bf16(r["h10"]).T, as_bf16(r["h11"]).T], axis=1)
         for r in r1], axis=0)

    r2 = run(lambda: _build_mp(cfg, plan, True),
             _pack_mp_inputs(cfg, plan, T1, W2p, b1, True), "out")
    # feature-major [128, NP] -> row-major table [NCORES*NP, 128]
    T2 = np.concatenate([as_bf16(r["out"]).T for r in r2], axis=0)

    r3 = run(lambda: _build_mp(cfg, plan, False),
             _pack_mp_inputs(cfg, plan, T2, None, bpp, False), "out")

    y = np.empty((cfg.N, cfg.OUT), np.float32)
    for k in range(cfg.NCORES):
        shard = as_bf16(r3[k]["out"]).T.astype(np.float32)   # [NP, OUT]
        y[plan.nodes[k]] = shard[:cfg.ND]
    return y, results


def kernel(**inputs):
    y, _ = kernel_run(inputs)
    return y


# revision 21
# speedup vs baseline: 1.1474x; 1.0076x over previous
"""Trainium2 Bass kernel: 2-layer GCN (GCNConv -> ReLU -> GCNConv -> Linear).

Strategy (8 NeuronCores, SPMD), v3 "dense edge-slab" design:
  - Destination-node sharding; nodes assigned to (core, lane) by a
    degree-sorted serpentine so per-(core,tile) edge counts match across
    cores (minimal static padding).
  - 3 launches with host-side exchange of the small activation tables:
      A: H1 = X @ W1                      (row-sharded dense matmul)
      B: MP1 + b1 + ReLU, then @ (W2 Wp) -> T2   (feature-major out)
      C: MP2 + bpp -> y                   (feature-major out)
  - Message passing consumes a host-expanded *dense edge slab*: for each
    128-edge chunk the 128 source rows are laid out contiguously in DRAM
    (edge order, dest-sorted, self-loops included as ordinary edges).  The
    device streams the slab at full DMA bandwidth -- no dma_gather, no
    GpSimd descriptor generation (the baseline bottleneck).
  - Transposed MP matmul: out[f, lane] += slab_chunk[slot, f]^T-contracted
    with wsl[slot, lane-window].  The destination window lives in the PSUM
    *free* dim, so any [mn..mx] window is legal (single matmul per chunk).
    The first chunk of each tile stores a full 128-wide weight block and
    runs with start=True (PSUM zeroing without a bias bracket).
  - Biases ride the Scalar-engine activation (per-partition bias AP), not
    PE matmuls.  Outputs are staged in SBUF and written in 8-tile batches.
  - All matmul operands bf16 (fp32 PSUM accumulation); final output fp32
    (bf16 on the wire, upcast on host).
"""

from contextlib import ExitStack
from dataclasses import dataclass, field

import numpy as np
import ml_dtypes

BF16 = ml_dtypes.bfloat16
E4M3 = ml_dtypes.float8_e4m3fn
FP32 = np.float32


# ---------------------------------------------------------------- config

@dataclass
class Cfg:
    N: int = 50000
    IN_DIM: int = 512
    HID: int = 256
    OUT: int = 128
    NCORES: int = 8
    BLK_B: int = 32       # slab chunks per stream DMA, launch B (16KB/part)
    BLK_C: int = 64       # launch C (16KB/part)
    TB_A: int = 8         # x tiles per stream DMA, launch A (8KB/part)
    GRP: int = 8          # output tiles per batched store
    F8_FRAC: float = 0.85     # target fraction of non-self edges in fp8

    ND: int = field(init=False)
    NTILES: int = field(init=False)
    NP: int = field(init=False)

    def __post_init__(self):
        self.ND = self.N // self.NCORES
        self.NTILES = (self.ND + 127) // 128
        self.NP = self.NTILES * 128


# ---------------------------------------------------------------- planner

class Plan:
    """Static (cross-core identical) chunk geometry + per-core data."""

    def __init__(self, cfg: Cfg, edge_index, edge_weight):
        self.cfg = cfg
        N, ND, NP, NT = cfg.N, cfg.ND, cfg.NP, cfg.NTILES
        NC = cfg.NCORES

        # --- gcn_norm with self loops (kept as ordinary edges)
        row = np.concatenate([np.asarray(edge_index[0], np.int64),
                              np.arange(N, dtype=np.int64)])
        col = np.concatenate([np.asarray(edge_index[1], np.int64),
                              np.arange(N, dtype=np.int64)])
        w = np.concatenate([np.asarray(edge_weight, np.float64),
                            np.ones(N, np.float64)])
        deg = np.zeros(N, np.float64)
        np.add.at(deg, col, w)
        dinv = np.where(deg > 0, 1.0 / np.sqrt(deg), 0.0)
        nrm = (dinv[row] * w * dinv[col]).astype(np.float32)

        # --- degree-sorted serpentine node -> (core, lane)
        degi = np.bincount(col, minlength=N)
        ranks = np.argsort(-degi, kind="stable")    # rank r -> node
        r = np.arange(N)
        blk = r // NC
        corepos = np.where(blk % 2 == 0, r % NC, NC - 1 - (r % NC))
        lane_global = np.empty(N, np.int64)         # node -> core*NP + lane
        lane_global[ranks] = corepos * NP + blk
        self.nodes = []                             # per core: lane -> node id
        for k in range(NC):
            nk = np.empty(ND, np.int64)
            sel = corepos == k
            nk[blk[sel]] = ranks[sel]
            self.nodes.append(nk)

        # --- edge geometry, dest-sorted
        src_row = lane_global[row]                  # table row of the source
        dst = lane_global[col]
        dst_core = dst // NP
        dlane = dst % NP
        dtile = dlane // 128
        dl = dlane % 128

        order = np.lexsort((dl, dtile, dst_core))
        so_core = dst_core[order]
        so_tile = dtile[order]
        so_lane = dl[order]
        so_src = src_row[order]
        so_w = nrm[order]

        # --- fp8 / bf16 split, chunk-aligned per (core,tile): every core
        # contributes exactly C8[t]*128 of its locally-lowest-|norm| non-self
        # edges to the fp8 stream, so the fp8 slab has zero padding.
        # Self-loops stay bf16 so every (core,tile) keeps a bf16 first chunk.
        so_self = self._selfmask(row, col, N, order)
        seg0 = so_core * NT + so_tile
        cnt0 = np.bincount(seg0, minlength=NC * NT).reshape(NC, NT)
        avail = cnt0 - 128                     # non-self edges per (core,tile)
        tgt = np.round(cfg.F8_FRAC * avail.mean(axis=0) / 128).astype(np.int64)
        C8t = np.minimum(tgt, (avail // 128).min(axis=0))
        C8t = np.maximum(C8t, 0)

        ordx = np.lexsort((np.abs(so_w), so_self, seg0))
        segstart = np.concatenate(
            [[0], np.cumsum(np.bincount(seg0, minlength=NC * NT))])[:-1]
        rank_in = np.arange(len(ordx)) - segstart[seg0[ordx]]
        take = rank_in < (C8t[so_tile[ordx]] * 128)
        f8 = np.zeros(len(so_w), bool)
        f8[ordx[take]] = True
        f8 &= ~so_self

        def geom(sel, force_first_full):
            seg_s = so_core[sel] * NT + so_tile[sel]
            cnt_s = np.bincount(seg_s, minlength=NC * NT).reshape(NC, NT)
            CH = (-(-cnt_s // 128)).max(axis=0)
            base = np.concatenate([[0], np.cumsum(CH)])
            TOT = int(base[-1])
            seg_start = np.concatenate(
                [[0], np.cumsum(np.bincount(seg_s, minlength=NC * NT))])[:-1]
            rank = np.arange(sel.sum()) - seg_start[seg_s]
            chunk = base[so_tile[sel]] + rank // 128
            slot = rank % 128
            mn = np.full(max(TOT, 1), 128, np.int64)
            mx = np.full(max(TOT, 1), -1, np.int64)
            lanes = so_lane[sel]
            np.minimum.at(mn, chunk, lanes)
            np.maximum.at(mx, chunk, lanes)
            empty = mx < 0
            mn[empty] = 0
            mx[empty] = mn[empty] - 1
            if force_first_full and TOT:
                first = base[:-1]
                mn[first] = 0
                mx[first] = 127
            span = mx - mn + 1
            off = np.concatenate([[0], np.cumsum(span)])
            return dict(CH=CH, base=base, TOT=TOT, chunk=chunk, slot=slot,
                        mn=mn, span=span, off=off,
                        SLAB=max(int(off[-1]), 1))

        gb = geom(~f8, True)
        g8 = geom(f8, False)
        self.gb, self.g8 = gb, g8
        self.TOTCH, self.SLAB = gb["TOT"], gb["SLAB"]
        self.TOTCH8, self.SLAB8 = max(g8["TOT"], 1), g8["SLAB"]

        # --- per-core arrays
        self.wsl = []       # bf16 one-hot*norm blocks [128, SLAB]
        self.pos = []       # bf16 edge -> slab position
        self.srcrow = []
        self.pos8 = []      # fp8 edge -> slab8 position
        self.srcrow8 = []
        self.w8 = []        # fp8 edge norms
        self.wcol8 = []     # fp8 edge -> (slot, wsl8 column)
        kb = so_core[~f8]
        k8 = so_core[f8]
        srcb, src8 = so_src[~f8], so_src[f8]
        laneb, lane8 = so_lane[~f8], so_lane[f8]
        wb, w8v = so_w[~f8], so_w[f8]
        for k in range(NC):
            m = kb == k
            kchunk, kslot = gb["chunk"][m], gb["slot"][m]
            wsl = np.zeros((128, self.SLAB), np.float32)
            wsl[kslot, gb["off"][kchunk] + (laneb[m] - gb["mn"][kchunk])] = wb[m]
            self.wsl.append(wsl.astype(BF16))
            self.pos.append(kslot * self.TOTCH + kchunk)
            self.srcrow.append(srcb[m])

            m8 = k8 == k
            kchunk8, kslot8 = g8["chunk"][m8], g8["slot"][m8]
            self.pos8.append(kslot8 * self.TOTCH8 + kchunk8)
            self.srcrow8.append(src8[m8])
            self.w8.append(w8v[m8].astype(np.float64))
            self.wcol8.append((kslot8,
                               g8["off"][kchunk8]
                               + (lane8[m8] - g8["mn"][kchunk8])))

    @staticmethod
    def _selfmask(row, col, N, order):
        m = np.zeros(len(row), bool)
        m[len(row) - N:] = True
        return m[order]

    @staticmethod
    def row_scales(tab: np.ndarray) -> np.ndarray:
        mx = np.maximum(np.abs(tab.astype(np.float32)).max(axis=1), 1e-20)
        return (240.0 / mx).astype(np.float32)

    def build_slab(self, k: int, tab: np.ndarray) -> np.ndarray:
        """Dense edge-ordered bf16 slab [128, TOTCH*F] for core k."""
        F = tab.shape[1]
        flat = np.zeros((128 * self.TOTCH, F), BF16)
        flat[self.pos[k]] = tab[self.srcrow[k]]
        return flat.reshape(128, self.TOTCH * F)

    def build_slab8(self, k: int, tab: np.ndarray, s: np.ndarray):
        """Dense edge-ordered fp8 slab [128, TOTCH8*F] (rows scaled by s)."""
        F = tab.shape[1]
        flat = np.zeros((128 * self.TOTCH8, F), E4M3)
        src = self.srcrow8[k]
        flat[self.pos8[k]] = (tab[src].astype(np.float32)
                              * s[src][:, None]).astype(E4M3)
        return flat.reshape(128, self.TOTCH8 * F)

    def build_wsl8(self, k: int, s: np.ndarray) -> np.ndarray:
        arr = np.zeros((128, self.SLAB8), np.float32)
        kslot, kcol = self.wcol8[k]
        arr[kslot, kcol] = self.w8[k] / s[self.srcrow8[k]]
        return arr.astype(BF16)


# ---------------------------------------------------------------- bass builders

def _build_l1(cfg: Cfg):
    """H1 = X @ W1, feature-major output (two halves h1a/h1b [128, NP]).
    xt lives fully in SBUF (12.5KB/part); few, large DMAs -- the SP
    sequencer's ~0.6us per dma_start was the previous bottleneck."""
    import concourse.bacc as bacc
    import concourse.mybir as mybir
    import concourse.tile as tile

    dt = mybir.dt
    nc = bacc.Bacc(None, target_bir_lowering=False, num_swdge_queues=4)
    KCH = cfg.IN_DIM // 128
    G = 4                                   # tiles per matmul (512 lanes)
    NG = -(-cfg.NTILES // G)
    L = G * 128
    SG = 4                                  # matmul groups per output stage
    # c-major x^T: block c is x[:, c*128:(c+1)*128].T laid out [128, NP]
    xt = nc.dram_tensor("xt", [128, KCH * cfg.NP], dt.bfloat16,
                        kind="ExternalInput")
    w1 = nc.dram_tensor("w1", [128, KCH * cfg.HID], dt.bfloat16,
                        kind="ExternalInput")
    outs_d = [nc.dram_tensor(f"h1{h}", [128, cfg.NP], dt.bfloat16,
                             kind="ExternalOutput") for h in range(2)]

    with tile.TileContext(nc) as tc, ExitStack() as ctx:
        consts = ctx.enter_context(tc.tile_pool(name="consts", bufs=1))
        stg = ctx.enter_context(tc.tile_pool(name="stg", bufs=2))
        pools = [ctx.enter_context(tc.tile_pool(name=f"psl{h}", bufs=3,
                                                space="PSUM"))
                 for h in range(2)]

        w1_sb = consts.tile([128, KCH * cfg.HID], dt.bfloat16, tag="w1")
        xt_sb = consts.tile([128, KCH * cfg.NP], dt.bfloat16, tag="xt")
        # interleave c-quarters so early (g, all-c) windows complete fast
        Q = 4
        qs = cfg.NP // Q
        nc.sync.dma_start(w1_sb[:], w1[:])
        for q in range(Q):
            for c in range(KCH):
                nc.sync.dma_start(
                    xt_sb[:, c * cfg.NP + q * qs: c * cfg.NP + (q + 1) * qs],
                    xt[:, c * cfg.NP + q * qs: c * cfg.NP + (q + 1) * qs])

        stages = [None, None]
        for g in range(NG):
            l0 = g * L
            l1 = min(cfg.NP, l0 + L)
            ll = l1 - l0
            sgi = g % SG
            if sgi == 0:
                nst = min(SG * L, cfg.NP - g * L)
                stages = [stg.tile([128, nst], dt.bfloat16, name="ostg")
                          for _ in range(2)]
            for h in range(2):
                ps = pools[h].tile([128, L], dt.float32, name="psl")
                for c in range(KCH):
                    nc.tensor.matmul(
                        ps[:, :ll],
                        w1_sb[:, c * cfg.HID + h * 128:
                              c * cfg.HID + (h + 1) * 128],
                        xt_sb[:, c * cfg.NP + l0: c * cfg.NP + l1],
                        start=(c == 0), stop=(c == KCH - 1),
                    )
                if h == 0:
                    nc.scalar.activation(
                        stages[h][:, sgi * L: sgi * L + ll], ps[:, :ll],
                        mybir.ActivationFunctionType.Copy)
                else:
                    nc.vector.tensor_copy(
                        stages[h][:, sgi * L: sgi * L + ll], ps[:, :ll])
            if sgi == SG - 1 or g == NG - 1:
                g0 = (g // SG) * SG * L
                for h in range(2):
                    nc.sync.dma_start(outs_d[h][:, g0:l1],
                                      stages[h][:, : l1 - g0])
    nc.finalize()
    return nc


def _build_mp(cfg: Cfg, plan: Plan, layer2: bool):
    """Transposed-MP launch.
    layer2: MP1 + b1 + ReLU + @W2p -> T2 [128, NP] bf16 (feature-major).
    else:   MP2 + bpp -> y [128, NP] bf16 (feature-major)."""
    import concourse.bacc as bacc
    import concourse.mybir as mybir
    import concourse.tile as tile

    dt = mybir.dt
    F = cfg.HID if layer2 else cfg.OUT          # slab feature width
    FCH = F // 128                              # psum column-tiles (2 or 1)
    BLK = cfg.BLK_B if layer2 else cfg.BLK_C
    BLK8 = 2 * BLK            # fp8 chunks are half the bytes
    GRP = cfg.GRP
    nc = bacc.Bacc(None, target_bir_lowering=False, num_swdge_queues=4)

    slab = nc.dram_tensor("slab", [128, plan.TOTCH * F], dt.bfloat16,
                          kind="ExternalInput")
    slab8 = nc.dram_tensor("slab8", [128, plan.TOTCH8 * F], dt.float8e4,
                           kind="ExternalInput")
    wsl = nc.dram_tensor("wsl", [128, plan.SLAB], dt.bfloat16,
                         kind="ExternalInput")
    wsl8 = nc.dram_tensor("wsl8", [128, plan.SLAB8], dt.bfloat16,
                          kind="ExternalInput")
    bvec = nc.dram_tensor("bvec", [128, FCH], dt.float32,
                          kind="ExternalInput")
    if layer2:
        w2p = nc.dram_tensor("w2p", [128, FCH * cfg.OUT], dt.bfloat16,
                             kind="ExternalInput")
    out = nc.dram_tensor("out", [128, cfg.NTILES * 128], dt.bfloat16,
                         kind="ExternalOutput")

    # split the wsl load at tile boundaries so early tiles' matmuls don't
    # wait on the whole slab-weight transfer
    nsplit = 4
    wcuts = [0]
    for i in range(1, nsplit):
        t = (cfg.NTILES * i) // nsplit
        wcuts.append(int(plan.gb["off"][plan.gb["base"][t]]))
    wcuts.append(plan.SLAB)

    with tile.TileContext(nc) as tc, ExitStack() as ctx:
        consts = ctx.enter_context(tc.tile_pool(name="consts", bufs=1))
        sstr = ctx.enter_context(tc.tile_pool(name="sstr", bufs=5))
        work = ctx.enter_context(tc.tile_pool(name="work", bufs=4))
        stg = ctx.enter_context(tc.tile_pool(name="stg", bufs=2))
        pools = [ctx.enter_context(tc.tile_pool(name=f"ps{c}",
                                                bufs=(3 if layer2 else 6),
                                                space="PSUM"))
                 for c in range(FCH)]
        if layer2:
            ps2p = ctx.enter_context(tc.tile_pool(name="ps2", bufs=2,
                                                  space="PSUM"))

        s8str = ctx.enter_context(tc.tile_pool(name="s8str", bufs=3))
        wsl_sb = consts.tile([128, plan.SLAB], dt.bfloat16, tag="wsl")
        wsl8_sb = consts.tile([128, plan.SLAB8], dt.bfloat16, tag="wsl8")
        bvec_sb = consts.tile([128, FCH], dt.float32, tag="bvec")
        if layer2:
            w2p_sb = consts.tile([128, FCH * cfg.OUT], dt.bfloat16, tag="w2p")

        stiles = {}

        def sblock(b):
            if b not in stiles:
                t = sstr.tile([128, BLK * F], dt.bfloat16, name="sb")
                c0 = b * BLK * F
                c1 = min(plan.TOTCH * F, c0 + BLK * F)
                nc.sync.dma_start(t[:, : c1 - c0], slab[:, c0:c1])
                stiles[b] = t
            return stiles[b]

        s8tiles = {}

        def s8block(b):
            if b not in s8tiles:
                t = s8str.tile([128, BLK8 * F], dt.float8e4, name="s8b")
                c0 = b * BLK8 * F
                c1 = min(plan.TOTCH8 * F, c0 + BLK8 * F)
                nc.sync.dma_start(t[:, : c1 - c0], slab8[:, c0:c1])
                s8tiles[b] = t
            return s8tiles[b]

        sblock(0)                 # slab block 0 first on the DMA queue
        for i in range(nsplit):
            nc.sync.dma_start(wsl_sb[:, wcuts[i]:wcuts[i + 1]],
                              wsl[:, wcuts[i]:wcuts[i + 1]])
        h8 = plan.SLAB8 // 2
        nc.sync.dma_start(wsl8_sb[:, :h8], wsl8[:, :h8])
        nc.sync.dma_start(wsl8_sb[:, h8:], wsl8[:, h8:])
        nc.sync.dma_start(bvec_sb[:], bvec[:])
        if layer2:
            nc.sync.dma_start(w2p_sb[:], w2p[:])

        stage = None
        for t in range(cfg.NTILES):
            g = t % GRP
            if g == 0:
                ntg = min(GRP, cfg.NTILES - t)
                stage = stg.tile([128, ntg * 128], dt.bfloat16,
                                 name="stage")
            pss = [pools[c].tile([128, 128], dt.float32, name=f"pst")
                   for c in range(FCH)]
            gb, g8 = plan.gb, plan.g8
            j0, j1 = int(gb["base"][t]), int(gb["base"][t + 1])
            e0, e1 = int(g8["base"][t]), int(g8["base"][t + 1])
            work_items = [(False, j) for j in range(j0, j1)
                          if gb["span"][j] > 0]
            work_items += [(True, j) for j in range(e0, e1)
                           if g8["span"][j] > 0]
            for wi, (is8, j) in enumerate(work_items):
                gg = g8 if is8 else gb
                sp = int(gg["span"][j])
                o0 = int(gg["off"][j])
                m0 = int(gg["mn"][j])
                if is8:
                    st = s8block(j // BLK8)
                    soff = (j % BLK8) * F
                    wtile = wsl8_sb
                else:
                    st = sblock(j // BLK)
                    soff = (j % BLK) * F
                    wtile = wsl_sb
                for c in range(FCH):
                    nc.tensor.matmul(
                        pss[c][:, m0:m0 + sp],
                        st[:, soff + c * 128: soff + (c + 1) * 128],
                        wtile[:, o0:o0 + sp],
                        start=(wi == 0), stop=(wi == len(work_items) - 1),
                        skip_group_check=True,
                    )

            if layer2:
                acts = []
                for c in range(FCH):
                    a = work.tile([128, 128], dt.bfloat16, name="act")
                    nc.scalar.activation(a[:], pss[c][:],
                                         mybir.ActivationFunctionType.Relu,
                                         bias=bvec_sb[:, c:c + 1])
                    acts.append(a)
                ps2 = ps2p.tile([128, cfg.OUT], dt.float32)
                for c in range(FCH):
                    nc.tensor.matmul(ps2[:],
                                     w2p_sb[:, c * cfg.OUT:(c + 1) * cfg.OUT],
                                     acts[c][:],
                                     start=(c == 0), stop=(c == FCH - 1))
                nc.scalar.activation(stage[:, g * 128:(g + 1) * 128], ps2[:],
                                     mybir.ActivationFunctionType.Copy)
            else:
                nc.scalar.add(stage[:, g * 128:(g + 1) * 128], pss[0][:],
                              bvec_sb[:, 0:1])

            if g == GRP - 1 or t == cfg.NTILES - 1:
                t0 = t - g
                nc.sync.dma_start(out[:, t0 * 128:(t + 1) * 128],
                                  stage[:, :(g + 1) * 128])

    nc.finalize()
    return nc


# ---------------------------------------------------------------- host packing

def _pack_l1_inputs(cfg: Cfg, plan: Plan, x, W1):
    KCH = cfg.IN_DIM // 128
    w1r = np.zeros((128, KCH * cfg.HID), BF16)
    for c in range(KCH):
        w1r[:, c * cfg.HID:(c + 1) * cfg.HID] = \
            W1[c * 128:(c + 1) * 128, :].astype(BF16)
    maps = []
    for k in range(cfg.NCORES):
        xs = np.zeros((cfg.NP, cfg.IN_DIM), np.float32)
        xs[:cfg.ND] = x[plan.nodes[k]]
        xtr = np.zeros((128, KCH * cfg.NP), BF16)
        for c in range(KCH):
            xtr[:, c * cfg.NP:(c + 1) * cfg.NP] = \
                xs[:, c * 128:(c + 1) * 128].T.astype(BF16)
        maps.append({"xt": xtr, "w1": w1r})
    return maps


def _pack_mp_inputs(cfg: Cfg, plan: Plan, table, Wn, b, layer2):
    F = cfg.HID if layer2 else cfg.OUT
    FCH = F // 128
    bvec = np.zeros((128, FCH), np.float32)
    for c in range(FCH):
        bvec[:, c] = b[c * 128:(c + 1) * 128]
    scales = Plan.row_scales(table)
    maps = []
    for k in range(cfg.NCORES):
        m = {
            "slab": plan.build_slab(k, table),
            "slab8": plan.build_slab8(k, table, scales),
            "wsl": plan.wsl[k],
            "wsl8": plan.build_wsl8(k, scales),
            "bvec": bvec,
        }
        if layer2:
            wnr = np.zeros((128, FCH * cfg.OUT), BF16)
            for c in range(FCH):
                wnr[:, c * cfg.OUT:(c + 1) * cfg.OUT] = \
                    Wn[c * 128:(c + 1) * 128, :].astype(BF16)
            m["w2p"] = wnr
        maps.append(m)
    return maps


# ---------------------------------------------------------------- driver

def _run(nc, in_maps, cfg, trace=False):
    from concourse.bass_utils import run_bass_kernel_spmd
    res = run_bass_kernel_spmd(nc, in_maps, list(range(cfg.NCORES)), trace=trace)
    return res


def kernel_run(inputs, cfg=None, trace=False, sim=False):
    cfg = cfg or Cfg()
    x = np.asarray(inputs["x"], np.float32)
    plan = Plan(cfg, np.asarray(inputs["edge_index"]),
                np.asarray(inputs["edge_weight"], np.float32))
    W1 = np.asarray(inputs["W1"], np.float32)
    b1 = np.asarray(inputs["b1"], np.float32)
    W2 = np.asarray(inputs["W2"], np.float32)
    b2 = np.asarray(inputs["b2"], np.float32)
    Wp = np.asarray(inputs["Wp"], np.float32)
    bp = np.asarray(inputs["bp"], np.float32)

    results = []

    def run(build, maps, outname):
        nc = build()
        if sim:
            from concourse.bass_interp import CoreSim
            outs = []
            for k in range(cfg.NCORES):
                s = CoreSim(nc)
                for name, arr in maps[k].items():
                    s.tensor(name)[:] = arr
                s.simulate()
                outs.append({outname: s.tensor(outname).copy()})
            results.append(None)
            return outs
        r = _run(nc, maps, cfg, trace=trace)
        results.append(r)
        return r.results

    # fold the post-projection into layer 2: A(relu1@W2)@Wp = A(relu1@(W2@Wp))
    W2p = (W2 @ Wp).astype(np.float32)
    bpp = (b2 @ Wp + bp).astype(np.float32)

    def as_bf16(a):
        a = np.asarray(a)
        return a if a.dtype == BF16 else a.view(BF16)

    r1 = run(lambda: _build_l1(cfg), _pack_l1_inputs(cfg, plan, x, W1), "h1")
    T1 = np.concatenate(
        [np.concatenate([as_bf16(r["h10"]).T, as_bf16(r["h11"]).T], axis=1)
         for r in r1], axis=0)

    r2 = run(lambda: _build_mp(cfg, plan, True),
             _pack_mp_inputs(cfg, plan, T1, W2p, b1, True), "out")
    # feature-major [128, NP] -> row-major table [NCORES*NP, 128]
    T2 = np.concatenate([as_bf16(r["out"]).T for r in r2], axis=0)

    r3 = run(lambda: _build_mp(cfg, plan, False),
             _pack_mp_inputs(cfg, plan, T2, None, bpp, False), "out")

    y = np.empty((cfg.N, cfg.OUT), np.float32)
    for k in range(cfg.NCORES):
        shard = as_bf16(r3[k]["out"]).T.astype(np.float32)   # [NP, OUT]
        y[plan.nodes[k]] = shard[:cfg.ND]
    return y, results


def kernel(**inputs):
    y, _ = kernel_run(inputs)
    return y
